# revision 26
# baseline (speedup 1.0000x reference)
"""BigBird attention kernel for 8 Trainium2 NeuronCores — sparse version.

Head-parallel sharding: core h computes head h end-to-end; the host sums the
8 partial output projections and adds the output bias.

Exploits the BigBird structure:

  allowed(q) = band(|q-k| <= 32)  ∪  global cols {0, S-1}  ∪  <=3 random cols

- Band: only the ~4 key-tiles overlapping each 256-query group are computed;
  out-of-band lanes are masked to -30 pre-exp with host-built predicate
  tiles, two key-tiles per DVE instruction.
- Global cols: one [2, 256] score strip per group; the two V rows enter the
  PSUM accumulation via a tiny 2-contract matmul; predicated masks de-dup
  the overlap with the band for the first/last group.
- Random cols (<=3 per query, host-verified): instead of gathering K/V on
  device (GPSIMD ap_gather costs ~100 cycles per 4 indices), the HOST
  gathers the x rows for each (query, slot) pair into xR [C, NR*S] (bf16)
  and the device projects them through a stacked [Wk | Wv] weight tile —
  one 4-chain matmul per 512 pairs yields the needed K and V columns in
  partitions 0:64 / 64:128 of PSUM. Sentinel slots point at column 0 and
  are killed by a -30 score offset (sval) accumulated into the score PSUM
  via a rank-1 matmul. Scores are per-column dot products (DVE multiply +
  PE ones-reduce broadcast over partitions).
- Global rows 0 / S-1 attend everywhere: a dedicated 2-query dense pass over
  all 32 key-tiles; its numer/denom overwrite those two output columns.

Bias simplifications (exact): bk shifts every score of a query row equally
-> softmax-invariant -> dropped. bv shifts the attention output uniformly
-> (attn+bv)@Wo^T = attn@Wo^T + bv@Wo^T -> folded into the host-side bias.

Main band pipeline is column-major (scores [k, q]); V row-tiles vS for the
AV matmuls are produced directly by a second row-major projection pass
(out[pos, d] via lhsT=x-tile) rather than PE transposes of V^T.

Shapes hardcoded for B=1, S=4096, C=512, H=8, Dh=64, fp32.
"""

import sys

import numpy as np

sys.path.insert(0, "/opt/trn_rl_repo")

B, S, C, H = 1, 4096, 512, 8
DH = C // H  # 64
G = 256  # query-group size
NG = S // G  # 16
NT = S // 128  # 32 key tiles
W = 32  # band half-width
NR = 3  # max random cols per query
NCH = 4  # query-groups per xR chunk
NIDX = NR * S  # 12288 (query, slot) pairs
CHW = NCH * NR * G  # 3072 pairs per chunk

_CACHE = {}


def _band_tiles(g):
    t0 = max(0, (G * g - W) // 128)
    t1 = min(NT - 1, (G * g + G - 1 + W) // 128)
    return t0, t1


def _build_bass(reps=1, ablate=frozenset()):
    """Build the per-head NEFF. reps>1 wraps the whole body in a hardware
    For_i loop that re-executes the identical kernel (same inputs, same
    outputs) reps times back-to-back — used by the benchmark harness to
    measure on-device per-execution time without host round trips.

    ablate: timing-only variants with named instruction groups skipped
    (outputs become garbage) — used to attribute HW time to kernel stages
    since NTFF profiling is unavailable here. Never set on the graded path.
    """
    import contextlib

    import concourse.bacc as bacc
    import concourse.mybir as mybir
    import concourse.tile as tile

    f32 = mybir.dt.float32
    f32r = mybir.dt.float32r
    bf16 = mybir.dt.bfloat16
    Exp = mybir.ActivationFunctionType.Exp
    Copy = mybir.ActivationFunctionType.Copy
    mult = mybir.AluOpType.mult
    add = mybir.AluOpType.add

    ab = lambda n: n in ablate

    nc = bacc.Bacc("TRN2", target_bir_lowering=False, debug=False)

    xT_d = nc.dram_tensor("xT", [C, S], f32r, kind="ExternalInput")
    wqT_d = nc.dram_tensor("wqT", [C, DH], f32r, kind="ExternalInput")
    wkT_d = nc.dram_tensor("wkT", [C, DH], f32r, kind="ExternalInput")
    wvT_d = nc.dram_tensor("wvT", [C, DH], f32r, kind="ExternalInput")
    wkv_d = nc.dram_tensor("wkv", [C, 2 * DH], bf16, kind="ExternalInput")
    woT_d = nc.dram_tensor("woT", [DH, C], f32r, kind="ExternalInput")
    bq8_d = nc.dram_tensor("bq8", [DH, 1], f32, kind="ExternalInput")
    xR_d = nc.dram_tensor("xR", [C, NIDX], bf16, kind="ExternalInput")
    sval_d = nc.dram_tensor("sval", [1, NIDX], bf16, kind="ExternalInput")
    id_d = nc.dram_tensor("identin", [128, 128], f32, kind="ExternalInput")
    mb_d = nc.dram_tensor("maskb", [128, 4, G], mybir.dt.uint8, kind="ExternalInput")
    ms_d = nc.dram_tensor("masks", [2, 2, G], mybir.dt.uint8, kind="ExternalInput")
    out_d = nc.dram_tensor("partial", [S, C], f32, kind="ExternalOutput")

    with tile.TileContext(nc) as tc:
        with (
            tc.For_i(0, reps) if reps > 1 else contextlib.nullcontext(),
            tc.tile_pool(name="const", bufs=1) as cpool,
            tc.tile_pool(name="big", bufs=1) as bigpool,
        ):
            ident = cpool.tile([128, 128], f32)
            nc.scalar.dma_start(out=ident, in_=id_d[:, :])
            maskb = cpool.tile([128, 4, G], mybir.dt.uint8, tag="maskb")
            nc.scalar.dma_start(out=maskb, in_=mb_d[:, :, :])
            masks = cpool.tile([2, 2, G], mybir.dt.uint8, tag="masks")
            nc.scalar.dma_start(out=masks, in_=ms_d[:, :, :])
            neg30 = cpool.tile([128, G], f32, tag="neg30")
            nc.vector.memset(neg30, -30.0)
            neg30b = cpool.tile([128, 2, G], f32, tag="neg30b")
            nc.vector.memset(neg30b, -30.0)
            # f32r tensors cannot be memset directly; stage via f32 + copy
            identr = cpool.tile([DH + 1, DH + 1], f32r, tag="identr")
            nc.vector.tensor_copy(identr, ident[0 : DH + 1, 0 : DH + 1])
            ones_f = cpool.tile([128, 1], f32, tag="ones_f")
            nc.vector.memset(ones_f, 1.0)
            onesb_f = cpool.tile([128, DH + 1], f32, tag="onesb_f")
            nc.vector.memset(onesb_f, 1.0)
            vones_f = cpool.tile([128, NT], f32, tag="vones_f")
            nc.vector.memset(vones_f, 1.0)
            ones_b = cpool.tile([128, DH + 1], f32r, tag="ones_b")
            nc.vector.tensor_copy(ones_b, onesb_f)
            ones_bf = cpool.tile([1, DH + 1], bf16, tag="ones_bf")
            nc.vector.tensor_copy(ones_bf, onesb_f[0:1, :])

            wq = cpool.tile([128, 4, DH], f32r, tag="wq")
            wk = cpool.tile([128, 4, DH], f32r, tag="wk")
            wv = cpool.tile([128, 4, DH], f32r, tag="wv")
            wkv = cpool.tile([128, 4, 2 * DH], bf16, tag="wkv")
            # weights/bias loads go on the scalar queue so the sync queue's
            # first transfer is the first x tile (PE starts sooner)
            nc.scalar.dma_start(out=wq, in_=wqT_d.rearrange("(a p) d -> p a d", p=128))
            nc.scalar.dma_start(out=wk, in_=wkT_d.rearrange("(a p) d -> p a d", p=128))
            nc.scalar.dma_start(out=wv, in_=wvT_d.rearrange("(a p) d -> p a d", p=128))
            nc.scalar.dma_start(out=wkv, in_=wkv_d.rearrange("(a p) d -> p a d", p=128))
            woT = cpool.tile([DH, C], f32r, tag="wo")
            nc.scalar.dma_start(out=woT, in_=woT_d[:, :])
            bq8 = cpool.tile([DH, 1], f32, tag="bq8")
            nc.scalar.dma_start(out=bq8, in_=bq8_d[:, :])
            sval = cpool.tile([1, NIDX], bf16, tag="sval")
            nc.scalar.dma_start(out=sval, in_=sval_d[:, :])

            # persistent per-head tensors
            qT = bigpool.tile([DH, S], f32r)  # Q^T / 8 applied via scale
            kT = bigpool.tile([DH, S], f32r)  # K^T (no bk: softmax-invariant)
            vS = bigpool.tile([128, NT, DH + 1], f32r)  # [V | ones] row-tiles
            dor_sb = bigpool.tile([DH + 1, 2], f32)  # rows-pass numer/denom
            q2 = bigpool.tile([DH, 2], f32r)
            k2 = bigpool.tile([DH, 2], f32r)
            v2 = bigpool.tile([DH, 2], f32)
            strip2v = bigpool.tile([2, DH + 1], f32r)

            nc.vector.tensor_copy(vS[:, :, DH : DH + 1], vones_f.unsqueeze(2))

            # Band lookahead: band scores for group g only need proj
            # groups <= g+1, so bands 0..PF-1 are computed inside the
            # projection loop (keeps PE dense across the phase boundary and
            # pulls the band Act/Pool load forward).
            PF = 8
            ptbig = bigpool.tile([128, PF + 1, 4, G], f32r)
            pt_ctr = [0]
            st_band = {}

            def band_stage(g, spspool):
                sl = slice(g * G, (g + 1) * G)
                t0, t1 = _band_tiles(g)
                nt = t1 - t0 + 1
                pt = ptbig[:, pt_ctr[0] % (PF + 1), :, :]
                pt_ctr[0] += 1
                # mask tile index a + moff: band offset delta = 128*t0 -
                # 256*g + 128*a = -128 + 128*(a + moff) for every group.
                # Mask + exp batched per PSUM-bank pair.
                moff = 1 if g == 0 else 0
                for a0 in range(0, 0 if ab("band") else nt, 2):
                    n2 = min(2, nt - a0)
                    sps = spspool.tile([128, 2, G], f32)
                    for a in range(a0, a0 + n2):
                        t = t0 + a
                        nc.tensor.matmul(
                            sps[:, a - a0, :],
                            kT[:, t * 128 : (t + 1) * 128],
                            qT[:, sl],
                            start=True,
                            stop=True,
                        )
                    if not ab("bandmask"):
                        nc.vector.copy_predicated(
                            sps[:, 0:n2, :],
                            maskb[:, moff + a0 : moff + a0 + n2, :],
                            neg30b[:, 0:n2, :],
                        )
                    if not ab("bandexp"):
                        nc.scalar.activation(
                            pt[:, a0 : a0 + n2, :], sps[:, 0:n2, :], Exp
                        )
                st_band[g] = (pt, nt, t0)

            # ---- phase 1: projections + band lookahead ----
            with (
                tc.tile_pool(name="xload", bufs=3) as xpool,
                tc.tile_pool(name="pjps", bufs=3, space="PSUM") as pjps,
                tc.tile_pool(name="sps1", bufs=2, space="PSUM") as sps1p,
                tc.tile_pool(name="vrow", bufs=2, space="PSUM") as vrowp,
                tc.tile_pool(name="v2p", bufs=1, space="PSUM") as v2pp,
            ):
                for g in range(NG):
                    sl = slice(g * G, (g + 1) * G)
                    xg = xpool.tile([128, 4, G], f32r)
                    if not ab("xdma"):
                        nc.sync.dma_start(
                            out=xg,
                            in_=xT_d.rearrange("(a p) s -> p a s", p=128)[:, :, sl],
                        )
                    ncb = 1 if ab("projlite") else 4
                    for wt, dst in ((wq, qT), (wk, kT)):
                        if ab("proj"):
                            break
                        pst = pjps.tile([128, G], f32)
                        ps = pst[0:DH, :]
                        for cb in range(ncb):
                            nc.tensor.matmul(
                                ps,
                                wt[:, cb, :],
                                xg[:, cb, :],
                                start=(cb == 0),
                                stop=(cb == ncb - 1),
                            )
                        if dst is qT:
                            nc.vector.tensor_scalar(
                                dst[:, sl], ps, 0.125, bq8, op0=mult, op1=add
                            )
                        else:
                            nc.vector.tensor_copy(dst[:, sl], ps)
                    # V row-tiles directly row-major: out[pos, d] via
                    # lhsT = x tile (c-contraction), rhs = Wv^T
                    if not ab("vrow"):
                        vps = vrowp.tile([128, 2, DH], f32, tag="vps")
                        for sub in range(2):
                            psl = slice(sub * 128, (sub + 1) * 128)
                            for cb in range(ncb):
                                nc.tensor.matmul(
                                    vps[:, sub, :],
                                    xg[:, cb, psl],
                                    wv[:, cb, :],
                                    start=(cb == 0),
                                    stop=(cb == ncb - 1),
                                )
                        nc.vector.tensor_copy(vS[:, 2 * g : 2 * g + 2, 0:DH], vps)
                    if 1 <= g <= PF:
                        band_stage(g - 1, sps1p)
                # V columns 0 / S-1 for the global-col strip (v2), straight
                # from x columns {0, S-1}
                x2 = xpool.tile([128, 4, 2], f32r, tag="x2")
                xTr = xT_d.rearrange("(a p) s -> p a s", p=128)
                nc.sync.dma_start(out=x2[:, :, 0:1], in_=xTr[:, :, 0:1])
                nc.sync.dma_start(out=x2[:, :, 1:2], in_=xTr[:, :, S - 1 : S])
                v2ps = v2pp.tile([DH, 2], f32, tag="v2ps")
                for cb in range(4):
                    nc.tensor.matmul(
                        v2ps,
                        wv[:, cb, :],
                        x2[:, cb, :],
                        start=(cb == 0),
                        stop=(cb == 3),
                    )
                nc.vector.tensor_copy(v2, v2ps)

            # small column extracts (global cols 0 and S-1)
            nc.vector.tensor_copy(q2[:, 0:1], qT[:, 0:1])
            nc.vector.tensor_copy(q2[:, 1:2], qT[:, S - 1 : S])
            nc.vector.tensor_copy(k2[:, 0:1], kT[:, 0:1])
            nc.vector.tensor_copy(k2[:, 1:2], kT[:, S - 1 : S])

            # ---- phase 2/3: global rows + main loop ----
            from contextlib import ExitStack

            with ExitStack() as stack:
                pool = lambda name, bufs, **kw: stack.enter_context(
                    tc.tile_pool(name=name, bufs=bufs, **kw)
                )
                # PSUM (8 banks x 2KB/partition):
                #   kvps 2x[128,512] = 2, sps 2x[128,2,G] = 2,
                #   exps 1x[128,4,G] = 2 (strip lives in slot 0),
                #   dot [128,G] + rps [128,1] = 1, o_ps [128,C] = 1
                kvpsp = pool("kvps", 2, space="PSUM")
                spsp = pool("sps2", 2, space="PSUM")
                expsp = pool("exps", 1, space="PSUM")
                dotp = pool("dot", 1, space="PSUM")
                opsp = pool("ops", 1, space="PSUM")
                # PSUM bank budget (bank-granular per tag-buf): kvps 2 +
                # sps 2 + exps 2 + dot(incl r_ps col) 1 + o_ps 1 = 8
                xrp = pool("xr", 2)
                krp = pool("kr", 2)
                vrp = pool("vr", 2)
                tmpp = pool("tmp", 2)
                p3sp = pool("p3s", 2)
                erp = pool("er", 2)
                pstp = pool("pst", 2)
                otp = pool("ot", 2)
                denp = pool("den", 2)
                rsbp = pool("rsb", 2)
                resp = pool("res", 2)

                # strip2v = [V[0]; V[S-1]] | ones  (via PE transpose of v2)
                rows_t = opsp.tile([128, C], f32, tag="o_ps")
                ps2v = rows_t[0:2, 128:192]
                nc.tensor.transpose(ps2v, v2, ident[:DH, :DH])
                nc.vector.tensor_copy(strip2v[:, 0:DH], ps2v)
                nc.vector.tensor_copy(strip2v[:, DH : DH + 1], ones_f[0:2, :])

                # global rows 0 / S-1: dense 2-query pass over all key tiles
                if not ab("rows"):
                    s2v = rows_t[:, 0:64].rearrange("p (t r) -> p t r", r=2)
                    for t in range(NT):
                        nc.tensor.matmul(
                            s2v[:, t, :],
                            kT[:, t * 128 : (t + 1) * 128],
                            q2,
                            start=True,
                            stop=True,
                        )
                    pt2 = pstp.tile([128, NT, 2], f32r, tag="rows")
                    nc.scalar.activation(
                        pt2, rows_t[:, 0:64].rearrange("p (t r) -> p t r", r=2), Exp
                    )
                    dor = rows_t[0 : DH + 1, 64:66]
                    for t in range(NT):
                        nc.tensor.matmul(
                            dor,
                            vS[:, t, :],
                            pt2[:, t, :],
                            start=(t == 0),
                            stop=(t == NT - 1),
                        )
                    nc.vector.tensor_copy(dor_sb, dor)

                chunk_state = {}
                st = {}

                def stage_a(g):
                    sl = slice(g * G, (g + 1) * G)

                    # xR chunk: project host-gathered x rows through the
                    # stacked [Wk | Wv] tile -> K cols in PSUM parts 0:64,
                    # V cols in 64:128; partition-shifted copies split them
                    # into bf16 SBUF staging.
                    if g % NCH == 0:
                        ch = g // NCH
                        kr_sb = krp.tile([DH, NCH * NR, G], bf16, name="kr_sb")
                        vr_sb = vrp.tile([DH, NCH * NR, G], bf16, name="vr_sb")
                        krf = kr_sb.rearrange("p a q -> p (a q)")
                        vrf = vr_sb.rearrange("p a q -> p (a q)")
                        for half in range(2):
                            xr = xrp.tile([128, 4, CHW // 2], bf16, name="xr")
                            if not ab("xr"):
                                nc.sync.dma_start(
                                    out=xr,
                                    in_=xR_d.rearrange("(a p) n -> p a n", p=128)[
                                        :,
                                        :,
                                        ch * CHW + half * (CHW // 2) : ch * CHW
                                        + (half + 1) * (CHW // 2),
                                    ],
                                )
                                for blk in range(3):
                                    kvps = kvpsp.tile([128, 512], f32, name="kvps")
                                    bsl = slice(blk * 512, (blk + 1) * 512)
                                    for cb in range(4):
                                        nc.tensor.matmul(
                                            kvps,
                                            wkv[:, cb, :],
                                            xr[:, cb, bsl],
                                            start=(cb == 0),
                                            stop=(cb == 3),
                                        )
                                    osl = slice(
                                        half * (CHW // 2) + blk * 512,
                                        half * (CHW // 2) + (blk + 1) * 512,
                                    )
                                    nc.vector.tensor_copy(krf[:, osl], kvps[0:DH, :])
                                    nc.vector.tensor_copy(
                                        vrf[:, osl], kvps[DH:128, :]
                                    )
                        chunk_state["kr"] = kr_sb
                        chunk_state["vr"] = vr_sb
                    ch = g // NCH
                    co = (g % NCH) * NR
                    kr = chunk_state["kr"][:, co : co + NR, :]
                    vr = chunk_state["vr"][:, co : co + NR, :]

                    exps = expsp.tile([128, 4, G], f32)
                    pstrip = pstp.tile([2, G], f32r)
                    if not ab("rnd"):
                        tmp = tmpp.tile([DH, NR, G], f32r)
                        for rr in range(NR):
                            nc.vector.tensor_mul(tmp[:, rr, :], kr[:, rr, :], qT[:, sl])
                        # per-column dot + partition broadcast in one matmul
                        # (all-ones [64, 65] lhsT), plus the sentinel -30
                        # offsets via a rank-1 accumulate; split at the PSUM
                        # bank boundary (slot 1 | slots 2:4)
                        so = ch * CHW + (g % NCH) * NR * G
                        for dsts, tsl, ssl in (
                            (exps[0 : DH + 1, 1, :], (0, 1), (so, so + G)),
                            (exps[0 : DH + 1, 2:4, :], (1, 3), (so + G, so + 3 * G)),
                        ):
                            nc.tensor.matmul(
                                dsts,
                                ones_b[0:DH, 0 : DH + 1],
                                tmp[:, tsl[0] : tsl[1], :],
                                start=True,
                                stop=False,
                            )
                            nc.tensor.matmul(
                                dsts,
                                ones_bf,
                                sval[0:1, ssl[0] : ssl[1]],
                                start=False,
                                stop=True,
                            )
                        p3b = p3sp.tile([DH + 1, NR, G], f32r)
                        nc.scalar.activation(p3b[:, 0, :], exps[0 : DH + 1, 1, :], Exp)
                        nc.scalar.activation(
                            p3b[:, 1:3, :], exps[0 : DH + 1, 2:4, :], Exp
                        )
                        er = erp.tile([DH + 1, NR, G], f32r)
                        nc.vector.tensor_mul(er[0:DH, :, :], vr, p3b[0:DH, :, :])
                        nc.vector.tensor_copy(
                            er[DH : DH + 1, :, :], p3b[DH : DH + 1, :, :]
                        )
                    else:
                        er = erp.tile([DH + 1, NR, G], f32r)

                    # late band groups (not prefetched during phase 1)
                    if g not in st_band:
                        band_stage(g, spsp)

                    # global-col strip (PSUM slot 0 of exps)
                    if not ab("strip"):
                        sps2 = exps[0:2, 0, :]
                        nc.tensor.matmul(sps2, k2, qT[:, sl], start=True, stop=True)
                        if g == 0:
                            # col 0 is in-band for q <= W: mask row 0 there
                            nc.vector.copy_predicated(
                                sps2, masks[:, 0, :], neg30[0:2, :]
                            )
                        if g == NG - 1:
                            # col S-1 is in-band for q >= S-1-W: mask row 1
                            nc.vector.copy_predicated(
                                sps2, masks[:, 1, :], neg30[0:2, :]
                            )
                        nc.scalar.activation(pstrip, sps2, Exp)
                    st[g] = (er, pstrip)

                def stage_b(g):
                    er, pstrip = st.pop(g)
                    pt, nt, t0 = st_band.pop(g)

                    # AV accumulation: band + strip + extras (via identity)
                    dot = dotp.tile([128, G + 1], f32, tag="dot")
                    do = dot[0 : DH + 1, 0:G]
                    av_ops = []
                    if not ab("avband"):
                        for a in range(nt):
                            av_ops.append((vS[:, t0 + a, :], pt[:, a, :]))
                    if not ab("avstrip"):
                        av_ops.append((strip2v, pstrip))
                    if not ab("avrnd"):
                        for rr in range(NR):
                            av_ops.append((identr, er[:, rr, :]))
                    for i, (lhs, rhs) in enumerate(av_ops):
                        nc.tensor.matmul(
                            do,
                            lhs,
                            rhs,
                            start=(i == 0),
                            stop=(i == len(av_ops) - 1),
                        )

                    # epilogue
                    if ab("epi"):
                        return
                    oTden = otp.tile([DH + 1, G], f32r)
                    nc.vector.tensor_copy(oTden, do)
                    if g == 0 and not ab("rows"):
                        nc.vector.tensor_copy(oTden[:, 0:1], dor_sb[:, 0:1])
                    if g == NG - 1 and not ab("rows"):
                        nc.vector.tensor_copy(oTden[:, G - 1 : G], dor_sb[:, 1:2])
                    den = denp.tile([DH + 1, G], f32)
                    nc.vector.reciprocal(den[DH : DH + 1, :], oTden[DH : DH + 1, :])
                    res = resp.tile([128, 2, C], f32)
                    r_ps = dot[:, G : G + 1]
                    for sub in range(G // 128):
                        ssl = slice(sub * 128, (sub + 1) * 128)
                        nc.tensor.transpose(
                            r_ps,
                            den[DH : DH + 1, ssl],
                            ones_f[DH : DH + 1, 0:1],
                        )
                        r_sb = rsbp.tile([128, 1], f32, tag=f"rsb{sub}")
                        nc.vector.tensor_copy(r_sb, r_ps)
                        o_ps = opsp.tile([128, C], f32, tag="o_ps")
                        nc.tensor.matmul(
                            o_ps,
                            oTden[0:DH, ssl],
                            woT,
                            start=True,
                            stop=True,
                        )
                        nc.scalar.activation(
                            res[:, sub, :], o_ps, Copy, bias=0.0, scale=r_sb
                        )
                    if not ab("odma"):
                        nc.sync.dma_start(
                            out=out_d[g * G : (g + 1) * G, :].rearrange(
                                "(s p) c -> p s c", p=128
                            ),
                            in_=res,
                        )

                stage_a(0)
                for g in range(NG):
                    if g + 1 < NG:
                        stage_a(g + 1)
                    stage_b(g)
    nc.compile()
    return nc


def _get_nc(reps=1):
    key = f"nc{reps}"
    if key not in _CACHE:
        _CACHE[key] = _build_bass(reps)
    return _CACHE[key]


def _make_in_maps(inp):
    import ml_dtypes

    x2 = np.asarray(inp["x"], dtype=np.float32).reshape(S, C)
    xT = np.ascontiguousarray(x2.T)
    m = np.asarray(inp["attn_mask"], dtype=bool)
    assert m.shape == (S, S)

    i = np.arange(S)
    band = np.abs(i[:, None] - i[None, :]) <= W
    # the kernel's structural assumptions, verified against the actual mask
    assert m[band].all(), "window not fully allowed"
    assert m[0, :].all() and m[-1, :].all(), "global rows missing"
    assert m[:, 0].all() and m[:, -1].all(), "global cols missing"
    ex = m & ~band
    ex[:, 0] = False
    ex[:, -1] = False
    ex[0, :] = False
    ex[-1, :] = False
    rows, cols = np.nonzero(ex)
    pos = np.arange(len(rows)) - np.searchsorted(rows, rows)
    assert len(rows) == 0 or pos.max() < NR, "more than NR extra cols in a row"
    idx_full = np.full((S, NR), S, np.int32)
    idx_full[rows, pos] = cols

    # pair n = g*NR*G + rr*G + q  ->  column idx_full[g*G+q, rr]
    idx_pairs = (
        idx_full.reshape(NG, G, NR).transpose(0, 2, 1).reshape(NIDX)
    )
    sentinel = idx_pairs == S
    cols_flat = np.where(sentinel, 0, idx_pairs)
    sval = np.where(sentinel, -30.0, 0.0).astype(ml_dtypes.bfloat16).reshape(1, NIDX)
    xR = np.ascontiguousarray(x2[cols_flat, :].T).astype(ml_dtypes.bfloat16)

    identin = np.eye(128, dtype=np.float32)
    # band mask tiles: M[i][p, f] = 1 where OUT of band for delta=-128+128*i
    maskb = np.zeros((128, 4, G), np.uint8)
    p_ = np.arange(128)[:, None]
    f_ = np.arange(G)[None, :]
    for ii in range(4):
        delta = -128 + 128 * ii
        maskb[:, ii, :] = (np.abs(delta + p_ - f_) > W).astype(np.uint8)
    masks = np.zeros((2, 2, G), np.uint8)
    masks[0, 0, :] = (np.arange(G) <= W)          # g=0 row 0: q <= W in band
    masks[1, 1, :] = (np.arange(G) >= G - 1 - W)  # g=15 row 1: q >= S-1-W
    Wq, Wk, Wv, Wo = (np.asarray(inp[k], np.float32) for k in ("Wq", "Wk", "Wv", "Wo"))
    bq = np.asarray(inp["bq"], np.float32)
    in_maps = []
    for h in range(H):
        hsl = slice(h * DH, (h + 1) * DH)
        wkv = np.hstack([Wk[hsl, :].T, Wv[hsl, :].T]).astype(ml_dtypes.bfloat16)
        in_maps.append(
            {
                "xT": xT,
                "xR": xR,
                "sval": sval,
                "identin": identin,
                "maskb": maskb,
                "masks": masks,
                "wqT": np.ascontiguousarray(Wq[hsl, :].T),
                "wkT": np.ascontiguousarray(Wk[hsl, :].T),
                "wvT": np.ascontiguousarray(Wv[hsl, :].T),
                "wkv": np.ascontiguousarray(wkv),
                "woT": np.ascontiguousarray(Wo[:, hsl].T),
                "bq8": bq[hsl].reshape(DH, 1) / 8.0,
            }
        )
    return in_maps


def _host_bias(inp):
    """bo plus the folded V-bias term: (attn + bv) @ Wo^T = attn @ Wo^T +
    bv @ Wo^T, summed per head."""
    Wo = np.asarray(inp["Wo"], np.float32)
    bv = np.asarray(inp["bv"], np.float32)
    bo = np.asarray(inp["bo"], np.float32)
    return bo + bv @ Wo.T


def kernel(x, attn_mask, Wq, bq, Wk, bk, Wv, bv, Wo, bo):
    from concourse.bass_utils import run_bass_kernel_spmd

    inp = dict(x=x, attn_mask=attn_mask, Wq=Wq, bq=bq, Wk=Wk, bk=bk,
               Wv=Wv, bv=bv, Wo=Wo, bo=bo)
    nc = _get_nc()
    in_maps = _make_in_maps(inp)
    bias = _host_bias(inp)
    for attempt in range(2):
        res = run_bass_kernel_spmd(nc, in_maps, core_ids=list(range(H)))
        acc = res.results[0]["partial"].astype(np.float64)
        for c in range(1, H):
            acc += res.results[c]["partial"]
        out = acc.astype(np.float32) + bias[None, :]
        # one retry on a non-finite flake (rare transient launch corruption)
        if np.isfinite(out).all():
            break
    return out.reshape(B, S, C)


# revision 29
# speedup vs baseline: 1.6843x; 1.6843x over previous
"""BigBird attention kernel for 8 Trainium2 NeuronCores — sparse version.

Head-parallel sharding: core h computes head h end-to-end; the host sums the
8 partial output projections and adds the output bias.

Exploits the BigBird structure:

  allowed(q) = band(|q-k| <= 32)  ∪  global cols {0, S-1}  ∪  <=3 random cols

- Band: only the ~4 key-tiles overlapping each 256-query group are computed;
  out-of-band lanes are masked to -30 pre-exp with host-built predicate
  tiles, two key-tiles per DVE instruction.
- Global cols: one [2, 256] score strip per group; the two V rows enter the
  PSUM accumulation via a tiny 2-contract matmul; predicated masks de-dup
  the overlap with the band for the first/last group.
- Random cols (<=3 per query, host-verified): instead of gathering K/V on
  device (GPSIMD ap_gather costs ~100 cycles per 4 indices), the HOST
  gathers the x rows for each (query, slot) pair into xR [C, NR*S] (bf16)
  and the device projects them through a stacked [Wk | Wv] weight tile —
  one 4-chain matmul per 512 pairs yields the needed K and V columns in
  partitions 0:64 / 64:128 of PSUM. Sentinel slots point at column 0 and
  are killed by a -30 score offset (sval) accumulated into the score PSUM
  via a rank-1 matmul. Scores are per-column dot products (DVE multiply +
  PE ones-reduce broadcast over partitions).
- Global rows 0 / S-1 attend everywhere: a dedicated 2-query dense pass over
  all 32 key-tiles; its numer/denom overwrite those two output columns.

Bias simplifications (exact): bk shifts every score of a query row equally
-> softmax-invariant -> dropped. bv shifts the attention output uniformly
-> (attn+bv)@Wo^T = attn@Wo^T + bv@Wo^T -> folded into the host-side bias.

Main band pipeline is column-major (scores [k, q]); V row-tiles vS for the
AV matmuls are produced directly by a second row-major projection pass
(out[pos, d] via lhsT=x-tile) rather than PE transposes of V^T.

Shapes hardcoded for B=1, S=4096, C=512, H=8, Dh=64, fp32.
"""

import sys

import numpy as np

sys.path.insert(0, "/opt/trn_rl_repo")

B, S, C, H = 1, 4096, 512, 8
DH = C // H  # 64
G = 256  # query-group size
NG = S // G  # 16
NT = S // 128  # 32 key tiles
W = 32  # band half-width
NR = 3  # max random cols per query
NCH = 4  # query-groups per xR chunk
NIDX = NR * S  # 12288 (query, slot) pairs
CHW = NCH * NR * G  # 3072 pairs per chunk

_CACHE = {}


def _band_tiles(g):
    t0 = max(0, (G * g - W) // 128)
    t1 = min(NT - 1, (G * g + G - 1 + W) // 128)
    return t0, t1


def _build_bass(reps=1, ablate=frozenset()):
    """Build the per-head NEFF. reps>1 wraps the whole body in a hardware
    For_i loop that re-executes the identical kernel (same inputs, same
    outputs) reps times back-to-back — used by the benchmark harness to
    measure on-device per-execution time without host round trips.

    ablate: timing-only variants with named instruction groups skipped
    (outputs become garbage) — used to attribute HW time to kernel stages
    since NTFF profiling is unavailable here. Never set on the graded path.
    """
    import contextlib

    import concourse.bacc as bacc
    import concourse.mybir as mybir
    import concourse.tile as tile

    f32 = mybir.dt.float32
    f32r = mybir.dt.float32r
    bf16 = mybir.dt.bfloat16
    Exp = mybir.ActivationFunctionType.Exp
    Copy = mybir.ActivationFunctionType.Copy
    mult = mybir.AluOpType.mult
    add = mybir.AluOpType.add

    ab = lambda n: n in ablate

    nc = bacc.Bacc("TRN2", target_bir_lowering=False, debug=False)

    xT_d = nc.dram_tensor("xT", [C, S], f32r, kind="ExternalInput")
    wqT_d = nc.dram_tensor("wqT", [C, DH], f32r, kind="ExternalInput")
    wkT_d = nc.dram_tensor("wkT", [C, DH], f32r, kind="ExternalInput")
    wvT_d = nc.dram_tensor("wvT", [C, DH], f32r, kind="ExternalInput")
    wkv_d = nc.dram_tensor("wkv", [C, 2 * DH], bf16, kind="ExternalInput")
    woT_d = nc.dram_tensor("woT", [DH, C], f32r, kind="ExternalInput")
    bq8_d = nc.dram_tensor("bq8", [DH, 1], f32, kind="ExternalInput")
    xR_d = nc.dram_tensor("xR", [C, NIDX], bf16, kind="ExternalInput")
    sval_d = nc.dram_tensor("sval", [1, NIDX], bf16, kind="ExternalInput")
    id_d = nc.dram_tensor("identin", [128, 128], f32, kind="ExternalInput")
    mb_d = nc.dram_tensor("maskm", [128, 4, G], f32r, kind="ExternalInput")
    ms_d = nc.dram_tensor("masks", [2, 2, G], mybir.dt.uint8, kind="ExternalInput")
    out_d = nc.dram_tensor("partial", [S, C], f32, kind="ExternalOutput")

    with tile.TileContext(nc) as tc:
        with (
            tc.For_i(0, reps) if reps > 1 else contextlib.nullcontext(),
            tc.tile_pool(name="const", bufs=1) as cpool,
            tc.tile_pool(name="big", bufs=1) as bigpool,
            tc.tile_pool(name="ptraw", bufs=2) as ptrawp,
        ):
            ident = cpool.tile([128, 128], f32)
            nc.scalar.dma_start(out=ident, in_=id_d[:, :])
            maskm = cpool.tile([128, 4, G], f32r, tag="maskm")
            nc.scalar.dma_start(out=maskm, in_=mb_d[:, :, :])
            masks = cpool.tile([2, 2, G], mybir.dt.uint8, tag="masks")
            nc.scalar.dma_start(out=masks, in_=ms_d[:, :, :])
            neg30 = cpool.tile([128, G], f32, tag="neg30")
            nc.vector.memset(neg30, -30.0)
            # f32r tensors cannot be memset directly; stage via f32 + copy
            identr = cpool.tile([DH + 1, DH + 1], f32r, tag="identr")
            nc.vector.tensor_copy(identr, ident[0 : DH + 1, 0 : DH + 1])
            ones_f = cpool.tile([128, 1], f32, tag="ones_f")
            nc.vector.memset(ones_f, 1.0)
            onesb_f = cpool.tile([128, DH + 1], f32, tag="onesb_f")
            nc.vector.memset(onesb_f, 1.0)
            vones_f = cpool.tile([128, NT], f32, tag="vones_f")
            nc.vector.memset(vones_f, 1.0)
            ones_b = cpool.tile([128, DH + 1], f32r, tag="ones_b")
            nc.vector.tensor_copy(ones_b, onesb_f)
            ones_bf = cpool.tile([1, DH + 1], bf16, tag="ones_bf")
            nc.vector.tensor_copy(ones_bf, onesb_f[0:1, :])

            wq = cpool.tile([128, 4, DH], f32r, tag="wq")
            wk = cpool.tile([128, 4, DH], f32r, tag="wk")
            wv = cpool.tile([128, 4, DH], f32r, tag="wv")
            wkv = cpool.tile([128, 4, 2 * DH], bf16, tag="wkv")
            # weights/bias loads go on the scalar queue so the sync queue's
            # first transfer is the first x tile (PE starts sooner)
            nc.scalar.dma_start(out=wq, in_=wqT_d.rearrange("(a p) d -> p a d", p=128))
            nc.scalar.dma_start(out=wk, in_=wkT_d.rearrange("(a p) d -> p a d", p=128))
            nc.scalar.dma_start(out=wv, in_=wvT_d.rearrange("(a p) d -> p a d", p=128))
            nc.scalar.dma_start(out=wkv, in_=wkv_d.rearrange("(a p) d -> p a d", p=128))
            woT = cpool.tile([DH, C], f32r, tag="wo")
            nc.scalar.dma_start(out=woT, in_=woT_d[:, :])
            bq8 = cpool.tile([DH, 1], f32, tag="bq8")
            nc.scalar.dma_start(out=bq8, in_=bq8_d[:, :])
            sval = cpool.tile([1, NIDX], bf16, tag="sval")
            nc.scalar.dma_start(out=sval, in_=sval_d[:, :])

            # persistent per-head tensors
            qT = bigpool.tile([DH, S], f32r)  # Q^T / 8 applied via scale
            kT = bigpool.tile([DH, S], f32r)  # K^T (no bk: softmax-invariant)
            vS = bigpool.tile([128, NT, DH + 1], f32r)  # [V | ones] row-tiles
            dor_sb = bigpool.tile([DH + 1, 2], f32)  # rows-pass numer/denom
            q2 = bigpool.tile([DH, 2], f32r)
            k2 = bigpool.tile([DH, 2], f32r)
            v2 = bigpool.tile([DH, 2], f32)
            strip2v = bigpool.tile([2, DH + 1], f32r)

            nc.vector.tensor_copy(vS[:, :, DH : DH + 1], vones_f.unsqueeze(2))

            # Band lookahead: band scores for group g only need proj
            # groups <= g+1, so bands 0..PF-1 are computed inside the
            # projection loop (keeps PE dense across the phase boundary and
            # pulls the band Act/Pool load forward).
            PF = 5
            ptbig = bigpool.tile([128, PF + 1, 4, G], f32r)
            pt_ctr = [0]
            st_band = {}

            def band_stage(g, spspool):
                sl = slice(g * G, (g + 1) * G)
                t0, t1 = _band_tiles(g)
                nt = t1 - t0 + 1
                pt = ptbig[:, pt_ctr[0] % (PF + 1), :, :]
                pt_ctr[0] += 1
                # mask tile index a + moff: band offset delta = 128*t0 -
                # 256*g + 128*a = -128 + 128*(a + moff) for every group.
                # Mask + exp batched per PSUM-bank pair.
                moff = 1 if g == 0 else 0
                ptraw = None
                for a0 in range(0, 0 if ab("band") else nt, 2):
                    n2 = min(2, nt - a0)
                    if ptraw is None:
                        ptraw = ptrawp.tile([128, 4, G], f32r, name="ptraw")
                    sps = spspool.tile([128, 2, G], f32)
                    for a in range(a0, a0 + n2):
                        t = t0 + a
                        nc.tensor.matmul(
                            sps[:, a - a0, :],
                            kT[:, t * 128 : (t + 1) * 128],
                            qT[:, sl],
                            start=True,
                            stop=True,
                        )
                    if not ab("bandexp"):
                        nc.scalar.activation(
                            ptraw[:, a0 : a0 + n2, :], sps[:, 0:n2, :], Exp
                        )
                if ptraw is not None and not ab("bandmask"):
                    # zero out-of-band lanes post-exp on the otherwise-idle
                    # GPSIMD engine (mask is 0/1 f32)
                    nc.gpsimd.tensor_mul(
                        pt[:, 0:nt, :],
                        ptraw[:, 0:nt, :],
                        maskm[:, moff : moff + nt, :],
                    )
                st_band[g] = (pt, nt, t0)

            # ---- phase 1: projections + band lookahead ----
            with (
                tc.tile_pool(name="xload", bufs=3) as xpool,
                tc.tile_pool(name="pjps", bufs=3, space="PSUM") as pjps,
                tc.tile_pool(name="sps1", bufs=2, space="PSUM") as sps1p,
                tc.tile_pool(name="vrow", bufs=2, space="PSUM") as vrowp,
                tc.tile_pool(name="v2p", bufs=1, space="PSUM") as v2pp,
            ):
                ncb = 1 if ab("projlite") else 4
                for g2 in range(NG // 2):
                    sl2 = slice(g2 * 2 * G, (g2 + 1) * 2 * G)
                    xg = xpool.tile([128, 4, 2 * G], f32r)
                    if not ab("xdma"):
                        nc.sync.dma_start(
                            out=xg,
                            in_=xT_d.rearrange("(a p) s -> p a s", p=128)[:, :, sl2],
                        )
                    for wt, dst in ((wq, qT), (wk, kT)):
                        if ab("proj"):
                            break
                        pst = pjps.tile([DH, 2 * G], f32)
                        for cb in range(ncb):
                            nc.tensor.matmul(
                                pst,
                                wt[:, cb, :],
                                xg[:, cb, :],
                                start=(cb == 0),
                                stop=(cb == ncb - 1),
                            )
                        if dst is qT:
                            nc.vector.tensor_scalar(
                                dst[:, sl2], pst, 0.125, bq8, op0=mult, op1=add
                            )
                        else:
                            nc.vector.tensor_copy(dst[:, sl2], pst)
                    # V row-tiles directly row-major: out[pos, d] via
                    # lhsT = x tile (c-contraction), rhs = Wv^T
                    if not ab("vrow"):
                        vps = vrowp.tile([128, 4, DH], f32, tag="vps")
                        for sub in range(4):
                            psl = slice(sub * 128, (sub + 1) * 128)
                            for cb in range(ncb):
                                nc.tensor.matmul(
                                    vps[:, sub, :],
                                    xg[:, cb, psl],
                                    wv[:, cb, :],
                                    start=(cb == 0),
                                    stop=(cb == ncb - 1),
                                )
                        nc.vector.tensor_copy(
                            vS[:, 4 * g2 : 4 * g2 + 4, 0:DH], vps
                        )
                    for g in (2 * g2, 2 * g2 + 1):
                        if 1 <= g <= PF:
                            band_stage(g - 1, sps1p)
                # V columns 0 / S-1 for the global-col strip (v2), straight
                # from x columns {0, S-1}
                x2 = xpool.tile([128, 4, 2], f32r, tag="x2")
                xTr = xT_d.rearrange("(a p) s -> p a s", p=128)
                nc.sync.dma_start(out=x2[:, :, 0:1], in_=xTr[:, :, 0:1])
                nc.sync.dma_start(out=x2[:, :, 1:2], in_=xTr[:, :, S - 1 : S])
                v2ps = v2pp.tile([DH, 2], f32, tag="v2ps")
                for cb in range(4):
                    nc.tensor.matmul(
                        v2ps,
                        wv[:, cb, :],
                        x2[:, cb, :],
                        start=(cb == 0),
                        stop=(cb == 3),
                    )
                nc.vector.tensor_copy(v2, v2ps)

            # small column extracts (global cols 0 and S-1)
            nc.vector.tensor_copy(q2[:, 0:1], qT[:, 0:1])
            nc.vector.tensor_copy(q2[:, 1:2], qT[:, S - 1 : S])
            nc.vector.tensor_copy(k2[:, 0:1], kT[:, 0:1])
            nc.vector.tensor_copy(k2[:, 1:2], kT[:, S - 1 : S])

            # ---- phase 2/3: global rows + main loop ----
            from contextlib import ExitStack

            with ExitStack() as stack:
                pool = lambda name, bufs, **kw: stack.enter_context(
                    tc.tile_pool(name=name, bufs=bufs, **kw)
                )
                # PSUM (8 banks x 2KB/partition):
                #   kvps 2x[128,512] = 2, sps 2x[128,2,G] = 2,
                #   exps 1x[128,4,G] = 2 (strip lives in slot 0),
                #   dot [128,G] + rps [128,1] = 1, o_ps [128,C] = 1
                kvpsp = pool("kvps", 2, space="PSUM")
                spsp = pool("sps2", 2, space="PSUM")
                expsp = pool("exps", 1, space="PSUM")
                dotp = pool("dot", 1, space="PSUM")
                opsp = pool("ops", 1, space="PSUM")
                # PSUM bank budget (bank-granular per tag-buf): kvps 2 +
                # sps 2 + exps 2 + dot(incl r_ps col) 1 + o_ps 1 = 8
                xrp = pool("xr", 2)
                krp = pool("kr", 2)
                vrp = pool("vr", 2)
                tmpp = pool("tmp", 2)
                p3sp = pool("p3s", 2)
                erp = pool("er", 2)
                pstp = pool("pst", 2)
                otp = pool("ot", 2)
                denp = pool("den", 2)
                rsbp = pool("rsb", 2)
                resp = pool("res", 2)

                # strip2v = [V[0]; V[S-1]] | ones  (via PE transpose of v2)
                rows_t = opsp.tile([128, C], f32, tag="o_ps")
                ps2v = rows_t[0:2, 128:192]
                nc.tensor.transpose(ps2v, v2, ident[:DH, :DH])
                nc.vector.tensor_copy(strip2v[:, 0:DH], ps2v)
                nc.vector.tensor_copy(strip2v[:, DH : DH + 1], ones_f[0:2, :])

                # global rows 0 / S-1: dense 2-query pass over all key tiles
                if not ab("rows"):
                    s2v = rows_t[:, 0:64].rearrange("p (t r) -> p t r", r=2)
                    for t in range(NT):
                        nc.tensor.matmul(
                            s2v[:, t, :],
                            kT[:, t * 128 : (t + 1) * 128],
                            q2,
                            start=True,
                            stop=True,
                        )
                    pt2 = pstp.tile([128, NT, 2], f32r, tag="rows")
                    nc.scalar.activation(
                        pt2, rows_t[:, 0:64].rearrange("p (t r) -> p t r", r=2), Exp
                    )
                    dor = rows_t[0 : DH + 1, 64:66]
                    for t in range(NT):
                        nc.tensor.matmul(
                            dor,
                            vS[:, t, :],
                            pt2[:, t, :],
                            start=(t == 0),
                            stop=(t == NT - 1),
                        )
                    nc.vector.tensor_copy(dor_sb, dor)

                chunk_state = {}
                res_state = {}
                st = {}

                def stage_a(g):
                    sl = slice(g * G, (g + 1) * G)

                    # xR chunk: project host-gathered x rows through the
                    # stacked [Wk | Wv] tile -> K cols in PSUM parts 0:64,
                    # V cols in 64:128; partition-shifted copies split them
                    # into bf16 SBUF staging.
                    if g % NCH == 0:
                        ch = g // NCH
                        kr_sb = krp.tile([DH, NCH * NR, G], bf16, name="kr_sb")
                        vr_sb = vrp.tile([DH + 1, NCH * NR, G], bf16, name="vr_sb")
                        nc.vector.tensor_copy(
                            vr_sb[DH : DH + 1, :, :].rearrange("p a q -> p (a q)"),
                            ones_f[0:1, 0:1].broadcast_to((1, NCH * NR * G)),
                        )
                        krf = kr_sb.rearrange("p a q -> p (a q)")
                        vrf = vr_sb[0:DH, :, :].rearrange("p a q -> p (a q)")
                        for half in range(2):
                            xr = xrp.tile([128, 4, CHW // 2], bf16, name="xr")
                            if not ab("xr"):
                                nc.sync.dma_start(
                                    out=xr,
                                    in_=xR_d.rearrange("(a p) n -> p a n", p=128)[
                                        :,
                                        :,
                                        ch * CHW + half * (CHW // 2) : ch * CHW
                                        + (half + 1) * (CHW // 2),
                                    ],
                                )
                                for blk in range(3):
                                    kvps = kvpsp.tile([128, 512], f32, name="kvps")
                                    bsl = slice(blk * 512, (blk + 1) * 512)
                                    for cb in range(4):
                                        nc.tensor.matmul(
                                            kvps,
                                            wkv[:, cb, :],
                                            xr[:, cb, bsl],
                                            start=(cb == 0),
                                            stop=(cb == 3),
                                        )
                                    osl = slice(
                                        half * (CHW // 2) + blk * 512,
                                        half * (CHW // 2) + (blk + 1) * 512,
                                    )
                                    nc.vector.tensor_copy(krf[:, osl], kvps[0:DH, :])
                                    nc.vector.tensor_copy(
                                        vrf[:, osl], kvps[DH:128, :]
                                    )
                        chunk_state["kr"] = kr_sb
                        chunk_state["vr"] = vr_sb
                    ch = g // NCH
                    co = (g % NCH) * NR
                    kr = chunk_state["kr"][:, co : co + NR, :]
                    vr = chunk_state["vr"][:, co : co + NR, :]  # [DH+1, NR, G] incl ones row

                    exps = expsp.tile([128, 4, G], f32)
                    pstrip = pstp.tile([2, G], f32r)
                    if not ab("rnd"):
                        tmp = tmpp.tile([DH, NR, G], f32r)
                        nc.vector.tensor_mul(
                            tmp,
                            kr,
                            qT[:, sl].unsqueeze(1).broadcast_to((DH, NR, G)),
                        )
                        # per-column dot + partition broadcast in one matmul
                        # (all-ones [64, 65] lhsT), plus the sentinel -30
                        # offsets via a rank-1 accumulate; split at the PSUM
                        # bank boundary (slot 1 | slots 2:4)
                        so = ch * CHW + (g % NCH) * NR * G
                        for dsts, tsl, ssl in (
                            (exps[0 : DH + 1, 1, :], (0, 1), (so, so + G)),
                            (exps[0 : DH + 1, 2:4, :], (1, 3), (so + G, so + 3 * G)),
                        ):
                            nc.tensor.matmul(
                                dsts,
                                ones_b[0:DH, 0 : DH + 1],
                                tmp[:, tsl[0] : tsl[1], :],
                                start=True,
                                stop=False,
                            )
                            nc.tensor.matmul(
                                dsts,
                                ones_bf,
                                sval[0:1, ssl[0] : ssl[1]],
                                start=False,
                                stop=True,
                            )
                        p3b = p3sp.tile([DH + 1, NR, G], f32r)
                        nc.scalar.activation(p3b[:, 0, :], exps[0 : DH + 1, 1, :], Exp)
                        nc.scalar.activation(
                            p3b[:, 1:3, :], exps[0 : DH + 1, 2:4, :], Exp
                        )
                        er = erp.tile([DH + 1, NR, G], f32r)
                        nc.vector.tensor_mul(er, vr, p3b)
                    else:
                        er = erp.tile([DH + 1, NR, G], f32r)

                    # late band groups (not prefetched during phase 1)
                    if g not in st_band:
                        band_stage(g, spsp)

                    # global-col strip (PSUM slot 0 of exps)
                    if not ab("strip"):
                        sps2 = exps[0:2, 0, :]
                        nc.tensor.matmul(sps2, k2, qT[:, sl], start=True, stop=True)
                        if g == 0:
                            # col 0 is in-band for q <= W: mask row 0 there
                            nc.vector.copy_predicated(
                                sps2, masks[:, 0, :], neg30[0:2, :]
                            )
                        if g == NG - 1:
                            # col S-1 is in-band for q >= S-1-W: mask row 1
                            nc.vector.copy_predicated(
                                sps2, masks[:, 1, :], neg30[0:2, :]
                            )
                        nc.scalar.activation(pstrip, sps2, Exp)
                    st[g] = (er, pstrip)

                def stage_b(g):
                    er, pstrip = st.pop(g)
                    pt, nt, t0 = st_band.pop(g)

                    # AV accumulation: band + strip + extras (via identity)
                    dot = dotp.tile([128, G + 1], f32, tag="dot")
                    do = dot[0 : DH + 1, 0:G]
                    av_ops = []
                    if not ab("avband"):
                        for a in range(nt):
                            av_ops.append((vS[:, t0 + a, :], pt[:, a, :]))
                    if not ab("avstrip"):
                        av_ops.append((strip2v, pstrip))
                    if not ab("avrnd"):
                        for rr in range(NR):
                            av_ops.append((identr, er[:, rr, :]))
                    for i, (lhs, rhs) in enumerate(av_ops):
                        nc.tensor.matmul(
                            do,
                            lhs,
                            rhs,
                            start=(i == 0),
                            stop=(i == len(av_ops) - 1),
                        )

                    # epilogue
                    if ab("epi"):
                        return
                    oTden = otp.tile([DH + 1, G], f32r)
                    nc.vector.tensor_copy(oTden, do)
                    if g == 0 and not ab("rows"):
                        nc.vector.tensor_copy(oTden[:, 0:1], dor_sb[:, 0:1])
                    if g == NG - 1 and not ab("rows"):
                        nc.vector.tensor_copy(oTden[:, G - 1 : G], dor_sb[:, 1:2])
                    den = denp.tile([DH + 1, G], f32)
                    nc.vector.reciprocal(den[DH : DH + 1, :], oTden[DH : DH + 1, :])
                    if g % 2 == 0:
                        res_state["res"] = resp.tile([128, 4, C], f32, name="res")
                    res = res_state["res"]
                    r_ps = dot[:, G : G + 1]
                    for sub in range(G // 128):
                        ssl = slice(sub * 128, (sub + 1) * 128)
                        nc.tensor.transpose(
                            r_ps,
                            den[DH : DH + 1, ssl],
                            ones_f[DH : DH + 1, 0:1],
                        )
                        r_sb = rsbp.tile([128, 1], f32, tag=f"rsb{sub}")
                        nc.vector.tensor_copy(r_sb, r_ps)
                        o_ps = opsp.tile([128, C], f32, tag="o_ps")
                        nc.tensor.matmul(
                            o_ps,
                            oTden[0:DH, ssl],
                            woT,
                            start=True,
                            stop=True,
                        )
                        nc.scalar.activation(
                            res[:, (g % 2) * 2 + sub, :], o_ps, Copy,
                            bias=0.0, scale=r_sb,
                        )
                    if g % 2 == 1 and not ab("odma"):
                        nc.sync.dma_start(
                            out=out_d[(g - 1) * G : (g + 1) * G, :].rearrange(
                                "(s p) c -> p s c", p=128
                            ),
                            in_=res,
                        )

                stage_a(0)
                for g in range(NG):
                    if g + 1 < NG:
                        stage_a(g + 1)
                    stage_b(g)
    nc.compile()
    return nc


def _get_nc(reps=1):
    key = f"nc{reps}"
    if key not in _CACHE:
        _CACHE[key] = _build_bass(reps)
    return _CACHE[key]


def _make_in_maps(inp):
    import ml_dtypes

    x2 = np.asarray(inp["x"], dtype=np.float32).reshape(S, C)
    xT = np.ascontiguousarray(x2.T)
    m = np.asarray(inp["attn_mask"], dtype=bool)
    assert m.shape == (S, S)

    i = np.arange(S)
    band = np.abs(i[:, None] - i[None, :]) <= W
    # the kernel's structural assumptions, verified against the actual mask
    assert m[band].all(), "window not fully allowed"
    assert m[0, :].all() and m[-1, :].all(), "global rows missing"
    assert m[:, 0].all() and m[:, -1].all(), "global cols missing"
    ex = m & ~band
    ex[:, 0] = False
    ex[:, -1] = False
    ex[0, :] = False
    ex[-1, :] = False
    rows, cols = np.nonzero(ex)
    pos = np.arange(len(rows)) - np.searchsorted(rows, rows)
    assert len(rows) == 0 or pos.max() < NR, "more than NR extra cols in a row"
    idx_full = np.full((S, NR), S, np.int32)
    idx_full[rows, pos] = cols

    # pair n = g*NR*G + rr*G + q  ->  column idx_full[g*G+q, rr]
    idx_pairs = (
        idx_full.reshape(NG, G, NR).transpose(0, 2, 1).reshape(NIDX)
    )
    sentinel = idx_pairs == S
    cols_flat = np.where(sentinel, 0, idx_pairs)
    sval = np.where(sentinel, -30.0, 0.0).astype(ml_dtypes.bfloat16).reshape(1, NIDX)
    xR = np.ascontiguousarray(x2[cols_flat, :].T).astype(ml_dtypes.bfloat16)

    identin = np.eye(128, dtype=np.float32)
    # band mask tiles: M[i][p, f] = 1.0 where IN band for delta=-128+128*i
    # (multiplied into exp(scores) on gpsimd; 0 kills out-of-band lanes)
    maskm = np.zeros((128, 4, G), np.float32)
    p_ = np.arange(128)[:, None]
    f_ = np.arange(G)[None, :]
    for ii in range(4):
        delta = -128 + 128 * ii
        maskm[:, ii, :] = (np.abs(delta + p_ - f_) <= W).astype(np.float32)
    masks = np.zeros((2, 2, G), np.uint8)
    masks[0, 0, :] = (np.arange(G) <= W)          # g=0 row 0: q <= W in band
    masks[1, 1, :] = (np.arange(G) >= G - 1 - W)  # g=15 row 1: q >= S-1-W
    Wq, Wk, Wv, Wo = (np.asarray(inp[k], np.float32) for k in ("Wq", "Wk", "Wv", "Wo"))
    bq = np.asarray(inp["bq"], np.float32)
    in_maps = []
    for h in range(H):
        hsl = slice(h * DH, (h + 1) * DH)
        wkv = np.hstack([Wk[hsl, :].T, Wv[hsl, :].T]).astype(ml_dtypes.bfloat16)
        in_maps.append(
            {
                "xT": xT,
                "xR": xR,
                "sval": sval,
                "identin": identin,
                "maskm": maskm,
                "masks": masks,
                "wqT": np.ascontiguousarray(Wq[hsl, :].T),
                "wkT": np.ascontiguousarray(Wk[hsl, :].T),
                "wvT": np.ascontiguousarray(Wv[hsl, :].T),
                "wkv": np.ascontiguousarray(wkv),
                "woT": np.ascontiguousarray(Wo[:, hsl].T),
                "bq8": bq[hsl].reshape(DH, 1) / 8.0,
            }
        )
    return in_maps


def _host_bias(inp):
    """bo plus the folded V-bias term: (attn + bv) @ Wo^T = attn @ Wo^T +
    bv @ Wo^T, summed per head."""
    Wo = np.asarray(inp["Wo"], np.float32)
    bv = np.asarray(inp["bv"], np.float32)
    bo = np.asarray(inp["bo"], np.float32)
    return bo + bv @ Wo.T


def kernel(x, attn_mask, Wq, bq, Wk, bk, Wv, bv, Wo, bo):
    from concourse.bass_utils import run_bass_kernel_spmd

    inp = dict(x=x, attn_mask=attn_mask, Wq=Wq, bq=bq, Wk=Wk, bk=bk,
               Wv=Wv, bv=bv, Wo=Wo, bo=bo)
    nc = _get_nc()
    in_maps = _make_in_maps(inp)
    bias = _host_bias(inp)
    for attempt in range(2):
        res = run_bass_kernel_spmd(nc, in_maps, core_ids=list(range(H)))
        acc = res.results[0]["partial"].astype(np.float64)
        for c in range(1, H):
            acc += res.results[c]["partial"]
        out = acc.astype(np.float32) + bias[None, :]
        # one retry on a non-finite flake (rare transient launch corruption)
        if np.isfinite(out).all():
            break
    return out.reshape(B, S, C)


# revision 31
# speedup vs baseline: 1.6891x; 1.0029x over previous
"""BigBird attention kernel for 8 Trainium2 NeuronCores — sparse version.

Head-parallel sharding: core h computes head h end-to-end; the host sums the
8 partial output projections and adds the output bias.

Exploits the BigBird structure:

  allowed(q) = band(|q-k| <= 32)  ∪  global cols {0, S-1}  ∪  <=3 random cols

- Band: only the ~4 key-tiles overlapping each 256-query group are computed;
  out-of-band lanes are masked to -30 pre-exp with host-built predicate
  tiles, two key-tiles per DVE instruction.
- Global cols: one [2, 256] score strip per group; the two V rows enter the
  PSUM accumulation via a tiny 2-contract matmul; predicated masks de-dup
  the overlap with the band for the first/last group.
- Random cols (<=3 per query, host-verified): instead of gathering K/V on
  device (GPSIMD ap_gather costs ~100 cycles per 4 indices), the HOST
  gathers the x rows for each (query, slot) pair into xR [C, NR*S] (bf16)
  and the device projects them through a stacked [Wk | Wv] weight tile —
  one 4-chain matmul per 512 pairs yields the needed K and V columns in
  partitions 0:64 / 64:128 of PSUM. Sentinel slots point at column 0 and
  are killed by a -30 score offset (sval) accumulated into the score PSUM
  via a rank-1 matmul. Scores are per-column dot products (DVE multiply +
  PE ones-reduce broadcast over partitions).
- Global rows 0 / S-1 attend everywhere: a dedicated 2-query dense pass over
  all 32 key-tiles; its numer/denom overwrite those two output columns.

Bias simplifications (exact): bk shifts every score of a query row equally
-> softmax-invariant -> dropped. bv shifts the attention output uniformly
-> (attn+bv)@Wo^T = attn@Wo^T + bv@Wo^T -> folded into the host-side bias.

Main band pipeline is column-major (scores [k, q]); V row-tiles vS for the
AV matmuls are produced directly by a second row-major projection pass
(out[pos, d] via lhsT=x-tile) rather than PE transposes of V^T.

Shapes hardcoded for B=1, S=4096, C=512, H=8, Dh=64, fp32.
"""

import sys

import numpy as np

sys.path.insert(0, "/opt/trn_rl_repo")

B, S, C, H = 1, 4096, 512, 8
DH = C // H  # 64
G = 256  # query-group size
NG = S // G  # 16
NT = S // 128  # 32 key tiles
W = 32  # band half-width
NR = 3  # max random cols per query
NCH = 4  # query-groups per xR chunk
NIDX = NR * S  # 12288 (query, slot) pairs
CHW = NCH * NR * G  # 3072 pairs per chunk

_CACHE = {}


def _band_tiles(g):
    t0 = max(0, (G * g - W) // 128)
    t1 = min(NT - 1, (G * g + G - 1 + W) // 128)
    return t0, t1


def _build_bass(reps=1, ablate=frozenset()):
    """Build the per-head NEFF. reps>1 wraps the whole body in a hardware
    For_i loop that re-executes the identical kernel (same inputs, same
    outputs) reps times back-to-back — used by the benchmark harness to
    measure on-device per-execution time without host round trips.

    ablate: timing-only variants with named instruction groups skipped
    (outputs become garbage) — used to attribute HW time to kernel stages
    since NTFF profiling is unavailable here. Never set on the graded path.
    """
    import contextlib

    import concourse.bacc as bacc
    import concourse.mybir as mybir
    import concourse.tile as tile

    f32 = mybir.dt.float32
    f32r = mybir.dt.float32r
    bf16 = mybir.dt.bfloat16
    Exp = mybir.ActivationFunctionType.Exp
    Copy = mybir.ActivationFunctionType.Copy
    mult = mybir.AluOpType.mult
    add = mybir.AluOpType.add

    ab = lambda n: n in ablate

    nc = bacc.Bacc("TRN2", target_bir_lowering=False, debug=False)

    xT_d = nc.dram_tensor("xT", [C, S], f32r, kind="ExternalInput")
    wqT_d = nc.dram_tensor("wqT", [C, DH], f32r, kind="ExternalInput")
    wkT_d = nc.dram_tensor("wkT", [C, DH], f32r, kind="ExternalInput")
    wvT_d = nc.dram_tensor("wvT", [C, DH], f32r, kind="ExternalInput")
    wkv_d = nc.dram_tensor("wkv", [C, 2 * DH], bf16, kind="ExternalInput")
    woT_d = nc.dram_tensor("woT", [DH, C], f32r, kind="ExternalInput")
    bq8_d = nc.dram_tensor("bq8", [DH, 1], f32, kind="ExternalInput")
    xR_d = nc.dram_tensor("xR", [C, NIDX], bf16, kind="ExternalInput")
    sval_d = nc.dram_tensor("sval", [1, NIDX], bf16, kind="ExternalInput")
    id_d = nc.dram_tensor("identin", [128, 128], f32, kind="ExternalInput")
    mb_d = nc.dram_tensor("maskm", [128, 4, G], f32r, kind="ExternalInput")
    ms_d = nc.dram_tensor("masks", [2, 2, G], mybir.dt.uint8, kind="ExternalInput")
    out_d = nc.dram_tensor("partial", [S, C], f32, kind="ExternalOutput")

    with tile.TileContext(nc) as tc:
        with (
            tc.For_i(0, reps) if reps > 1 else contextlib.nullcontext(),
            tc.tile_pool(name="const", bufs=1) as cpool,
            tc.tile_pool(name="big", bufs=1) as bigpool,
            tc.tile_pool(name="ptraw", bufs=2) as ptrawp,
        ):
            ident = cpool.tile([128, 128], f32)
            nc.scalar.dma_start(out=ident, in_=id_d[:, :])
            maskm = cpool.tile([128, 4, G], f32r, tag="maskm")
            nc.scalar.dma_start(out=maskm, in_=mb_d[:, :, :])
            masks = cpool.tile([2, 2, G], mybir.dt.uint8, tag="masks")
            nc.scalar.dma_start(out=masks, in_=ms_d[:, :, :])
            neg30 = cpool.tile([128, G], f32, tag="neg30")
            nc.vector.memset(neg30, -30.0)
            # f32r tensors cannot be memset directly; stage via f32 + copy
            identr = cpool.tile([DH + 1, DH + 1], f32r, tag="identr")
            nc.vector.tensor_copy(identr, ident[0 : DH + 1, 0 : DH + 1])
            ones_f = cpool.tile([128, 1], f32, tag="ones_f")
            nc.vector.memset(ones_f, 1.0)
            onesb_f = cpool.tile([128, DH + 1], f32, tag="onesb_f")
            nc.vector.memset(onesb_f, 1.0)
            vones_f = cpool.tile([128, NT], f32, tag="vones_f")
            nc.vector.memset(vones_f, 1.0)
            ones_b = cpool.tile([128, DH + 1], f32r, tag="ones_b")
            nc.vector.tensor_copy(ones_b, onesb_f)
            ones_bf = cpool.tile([1, DH + 1], bf16, tag="ones_bf")
            nc.vector.tensor_copy(ones_bf, onesb_f[0:1, :])

            wq = cpool.tile([128, 4, DH], f32r, tag="wq")
            wk = cpool.tile([128, 4, DH], f32r, tag="wk")
            wv = cpool.tile([128, 4, DH], f32r, tag="wv")
            wkv = cpool.tile([128, 4, 2 * DH], bf16, tag="wkv")
            # weights/bias loads go on the scalar queue so the sync queue's
            # first transfer is the first x tile (PE starts sooner)
            nc.scalar.dma_start(out=wq, in_=wqT_d.rearrange("(a p) d -> p a d", p=128))
            nc.scalar.dma_start(out=wk, in_=wkT_d.rearrange("(a p) d -> p a d", p=128))
            nc.scalar.dma_start(out=wv, in_=wvT_d.rearrange("(a p) d -> p a d", p=128))
            nc.scalar.dma_start(out=wkv, in_=wkv_d.rearrange("(a p) d -> p a d", p=128))
            woT = cpool.tile([DH, C], f32r, tag="wo")
            nc.scalar.dma_start(out=woT, in_=woT_d[:, :])
            bq8 = cpool.tile([DH, 1], f32, tag="bq8")
            nc.scalar.dma_start(out=bq8, in_=bq8_d[:, :])
            sval = cpool.tile([1, NIDX], bf16, tag="sval")
            nc.scalar.dma_start(out=sval, in_=sval_d[:, :])

            # persistent per-head tensors
            qT = bigpool.tile([DH, S], f32r)  # Q^T / 8 applied via scale
            kT = bigpool.tile([DH, S], f32r)  # K^T (no bk: softmax-invariant)
            vS = bigpool.tile([128, NT, DH + 1], f32r)  # [V | ones] row-tiles
            dor_sb = bigpool.tile([DH + 1, 2], f32)  # rows-pass numer/denom
            q2 = bigpool.tile([DH, 2], f32r)
            k2 = bigpool.tile([DH, 2], f32r)
            v2 = bigpool.tile([DH, 2], f32)
            strip2v = bigpool.tile([2, DH + 1], f32r)

            nc.vector.tensor_copy(vS[:, :, DH : DH + 1], vones_f.unsqueeze(2))

            # Band lookahead: band scores for group g only need proj
            # groups <= g+1, so bands 0..PF-1 are computed inside the
            # projection loop (keeps PE dense across the phase boundary and
            # pulls the band Act/Pool load forward).
            PF = 5
            ptbig = bigpool.tile([128, PF + 1, 4, G], f32r)
            pt_ctr = [0]
            st_band = {}

            def band_stage(g, spspool):
                sl = slice(g * G, (g + 1) * G)
                t0, t1 = _band_tiles(g)
                nt = t1 - t0 + 1
                pt = ptbig[:, pt_ctr[0] % (PF + 1), :, :]
                pt_ctr[0] += 1
                # mask tile index a + moff: band offset delta = 128*t0 -
                # 256*g + 128*a = -128 + 128*(a + moff) for every group.
                # Mask + exp batched per PSUM-bank pair.
                moff = 1 if g == 0 else 0
                ptraw = None
                for a0 in range(0, 0 if ab("band") else nt, 2):
                    n2 = min(2, nt - a0)
                    if ptraw is None:
                        ptraw = ptrawp.tile([128, 4, G], f32r, name="ptraw")
                    sps = spspool.tile([128, 2, G], f32)
                    for a in range(a0, a0 + n2):
                        t = t0 + a
                        nc.tensor.matmul(
                            sps[:, a - a0, :],
                            kT[:, t * 128 : (t + 1) * 128],
                            qT[:, sl],
                            start=True,
                            stop=True,
                        )
                    if not ab("bandexp"):
                        nc.scalar.activation(
                            ptraw[:, a0 : a0 + n2, :], sps[:, 0:n2, :], Exp
                        )
                if ptraw is not None and not ab("bandmask"):
                    # zero out-of-band lanes post-exp on the otherwise-idle
                    # GPSIMD engine (mask is 0/1 f32)
                    nc.gpsimd.tensor_mul(
                        pt[:, 0:nt, :],
                        ptraw[:, 0:nt, :],
                        maskm[:, moff : moff + nt, :],
                    )
                st_band[g] = (pt, nt, t0)

            # ---- phase 1: projections + band lookahead ----
            with (
                tc.tile_pool(name="xload", bufs=3) as xpool,
                tc.tile_pool(name="pjps", bufs=3, space="PSUM") as pjps,
                tc.tile_pool(name="sps1", bufs=2, space="PSUM") as sps1p,
                tc.tile_pool(name="vrow", bufs=2, space="PSUM") as vrowp,
                tc.tile_pool(name="v2p", bufs=1, space="PSUM") as v2pp,
            ):
                ncb = 1 if ab("projlite") else 4
                for g2 in range(NG // 2):
                    sl2 = slice(g2 * 2 * G, (g2 + 1) * 2 * G)
                    xg = xpool.tile([128, 4, 2 * G], f32r)
                    if not ab("xdma"):
                        nc.sync.dma_start(
                            out=xg,
                            in_=xT_d.rearrange("(a p) s -> p a s", p=128)[:, :, sl2],
                        )
                    for wt, dst in ((wq, qT), (wk, kT)):
                        if ab("proj"):
                            break
                        pst = pjps.tile([DH, 2 * G], f32)
                        for cb in range(ncb):
                            nc.tensor.matmul(
                                pst,
                                wt[:, cb, :],
                                xg[:, cb, :],
                                start=(cb == 0),
                                stop=(cb == ncb - 1),
                            )
                        if dst is qT:
                            nc.vector.tensor_scalar(
                                dst[:, sl2], pst, 0.125, bq8, op0=mult, op1=add
                            )
                        else:
                            nc.vector.tensor_copy(dst[:, sl2], pst)
                    # V row-tiles directly row-major: out[pos, d] via
                    # lhsT = x tile (c-contraction), rhs = Wv^T
                    if not ab("vrow"):
                        vps = vrowp.tile([128, 4, DH], f32, tag="vps")
                        for sub in range(4):
                            psl = slice(sub * 128, (sub + 1) * 128)
                            for cb in range(ncb):
                                nc.tensor.matmul(
                                    vps[:, sub, :],
                                    xg[:, cb, psl],
                                    wv[:, cb, :],
                                    start=(cb == 0),
                                    stop=(cb == ncb - 1),
                                )
                        nc.vector.tensor_copy(
                            vS[:, 4 * g2 : 4 * g2 + 4, 0:DH], vps
                        )
                    for g in (2 * g2, 2 * g2 + 1):
                        if 1 <= g <= PF:
                            band_stage(g - 1, sps1p)
                # V columns 0 / S-1 for the global-col strip (v2), straight
                # from x columns {0, S-1}
                x2 = xpool.tile([128, 4, 2], f32r, tag="x2")
                xTr = xT_d.rearrange("(a p) s -> p a s", p=128)
                nc.sync.dma_start(out=x2[:, :, 0:1], in_=xTr[:, :, 0:1])
                nc.sync.dma_start(out=x2[:, :, 1:2], in_=xTr[:, :, S - 1 : S])
                v2ps = v2pp.tile([DH, 2], f32, tag="v2ps")
                for cb in range(4):
                    nc.tensor.matmul(
                        v2ps,
                        wv[:, cb, :],
                        x2[:, cb, :],
                        start=(cb == 0),
                        stop=(cb == 3),
                    )
                nc.vector.tensor_copy(v2, v2ps)

            # small column extracts (global cols 0 and S-1)
            nc.vector.tensor_copy(q2[:, 0:1], qT[:, 0:1])
            nc.vector.tensor_copy(q2[:, 1:2], qT[:, S - 1 : S])
            nc.vector.tensor_copy(k2[:, 0:1], kT[:, 0:1])
            nc.vector.tensor_copy(k2[:, 1:2], kT[:, S - 1 : S])

            # ---- phase 2/3: global rows + main loop ----
            from contextlib import ExitStack

            with ExitStack() as stack:
                pool = lambda name, bufs, **kw: stack.enter_context(
                    tc.tile_pool(name=name, bufs=bufs, **kw)
                )
                # PSUM (8 banks x 2KB/partition):
                #   kvps 2x[128,512] = 2, sps 2x[128,2,G] = 2,
                #   exps 1x[128,4,G] = 2 (strip lives in slot 0),
                #   dot [128,G] + rps [128,1] = 1, o_ps [128,C] = 1
                kvpsp = pool("kvps", 2, space="PSUM")
                spsp = pool("sps2", 2, space="PSUM")
                expsp = pool("exps", 1, space="PSUM")
                dotp = pool("dot", 1, space="PSUM")
                opsp = pool("ops", 1, space="PSUM")
                # PSUM bank budget (bank-granular per tag-buf): kvps 2 +
                # sps 2 + exps 2 + dot(incl r_ps col) 1 + o_ps 1 = 8
                xrp = pool("xr", 2)
                krp = pool("kr", 2)
                vrp = pool("vr", 2)
                tmpp = pool("tmp", 2)
                p3sp = pool("p3s", 2)
                erp = pool("er", 2)
                pstp = pool("pst", 2)
                otp = pool("ot", 2)
                denp = pool("den", 2)
                rsbp = pool("rsb", 2)
                resp = pool("res", 2)

                # strip2v = [V[0]; V[S-1]] | ones  (via PE transpose of v2)
                rows_t = opsp.tile([128, C], f32, tag="o_ps")
                ps2v = rows_t[0:2, 128:192]
                nc.tensor.transpose(ps2v, v2, ident[:DH, :DH])
                nc.vector.tensor_copy(strip2v[:, 0:DH], ps2v)
                nc.vector.tensor_copy(strip2v[:, DH : DH + 1], ones_f[0:2, :])

                # global rows 0 / S-1: dense 2-query pass over all key tiles
                if not ab("rows"):
                    s2v = rows_t[:, 0:64].rearrange("p (t r) -> p t r", r=2)
                    for t in range(NT):
                        nc.tensor.matmul(
                            s2v[:, t, :],
                            kT[:, t * 128 : (t + 1) * 128],
                            q2,
                            start=True,
                            stop=True,
                        )
                    pt2 = pstp.tile([128, NT, 2], f32r, tag="rows")
                    nc.scalar.activation(
                        pt2, rows_t[:, 0:64].rearrange("p (t r) -> p t r", r=2), Exp
                    )
                    dor = rows_t[0 : DH + 1, 64:66]
                    for t in range(NT):
                        nc.tensor.matmul(
                            dor,
                            vS[:, t, :],
                            pt2[:, t, :],
                            start=(t == 0),
                            stop=(t == NT - 1),
                        )
                    nc.vector.tensor_copy(dor_sb, dor)

                chunk_state = {}
                res_state = {}
                st = {}

                def stage_a(g):
                    sl = slice(g * G, (g + 1) * G)

                    # xR chunk: project host-gathered x rows through the
                    # stacked [Wk | Wv] tile -> K cols in PSUM parts 0:64,
                    # V cols in 64:128; partition-shifted copies split them
                    # into bf16 SBUF staging.
                    if g % NCH == 0:
                        ch = g // NCH
                        kr_sb = krp.tile([DH, NCH * NR, G], bf16, name="kr_sb")
                        vr_sb = vrp.tile([DH + 1, NCH * NR, G], bf16, name="vr_sb")
                        nc.vector.tensor_copy(
                            vr_sb[DH : DH + 1, :, :].rearrange("p a q -> p (a q)"),
                            ones_f[0:1, 0:1].broadcast_to((1, NCH * NR * G)),
                        )
                        krf = kr_sb.rearrange("p a q -> p (a q)")
                        vrf = vr_sb[0:DH, :, :].rearrange("p a q -> p (a q)")
                        for half in range(2):
                            xr = xrp.tile([128, 4, CHW // 2], bf16, name="xr")
                            if not ab("xr"):
                                nc.sync.dma_start(
                                    out=xr,
                                    in_=xR_d.rearrange("(a p) n -> p a n", p=128)[
                                        :,
                                        :,
                                        ch * CHW + half * (CHW // 2) : ch * CHW
                                        + (half + 1) * (CHW // 2),
                                    ],
                                )
                                for blk in range(3):
                                    kvps = kvpsp.tile([128, 512], f32, name="kvps")
                                    bsl = slice(blk * 512, (blk + 1) * 512)
                                    for cb in range(4):
                                        nc.tensor.matmul(
                                            kvps,
                                            wkv[:, cb, :],
                                            xr[:, cb, bsl],
                                            start=(cb == 0),
                                            stop=(cb == 3),
                                        )
                                    osl = slice(
                                        half * (CHW // 2) + blk * 512,
                                        half * (CHW // 2) + (blk + 1) * 512,
                                    )
                                    nc.vector.tensor_copy(krf[:, osl], kvps[0:DH, :])
                                    nc.vector.tensor_copy(
                                        vrf[:, osl], kvps[DH:128, :]
                                    )
                        chunk_state["kr"] = kr_sb
                        chunk_state["vr"] = vr_sb
                    ch = g // NCH
                    co = (g % NCH) * NR
                    kr = chunk_state["kr"][:, co : co + NR, :]
                    vr = chunk_state["vr"][:, co : co + NR, :]  # [DH+1, NR, G] incl ones row

                    exps = expsp.tile([128, 4, G], f32)
                    pstrip = pstp.tile([2, G], f32r)
                    if not ab("rnd"):
                        tmp = tmpp.tile([DH, NR, G], f32r)
                        nc.vector.tensor_mul(
                            tmp,
                            kr,
                            qT[:, sl].unsqueeze(1).broadcast_to((DH, NR, G)),
                        )
                        # per-column dot + partition broadcast in one matmul
                        # (all-ones [64, 65] lhsT), plus the sentinel -30
                        # offsets via a rank-1 accumulate; split at the PSUM
                        # bank boundary (slot 1 | slots 2:4)
                        so = ch * CHW + (g % NCH) * NR * G
                        for dsts, tsl, ssl in (
                            (exps[0 : DH + 1, 1, :], (0, 1), (so, so + G)),
                            (exps[0 : DH + 1, 2:4, :], (1, 3), (so + G, so + 3 * G)),
                        ):
                            nc.tensor.matmul(
                                dsts,
                                ones_b[0:DH, 0 : DH + 1],
                                tmp[:, tsl[0] : tsl[1], :],
                                start=True,
                                stop=False,
                            )
                            nc.tensor.matmul(
                                dsts,
                                ones_bf,
                                sval[0:1, ssl[0] : ssl[1]],
                                start=False,
                                stop=True,
                            )
                        p3b = p3sp.tile([DH + 1, NR, G], f32r)
                        nc.scalar.activation(p3b[:, 0, :], exps[0 : DH + 1, 1, :], Exp)
                        nc.scalar.activation(
                            p3b[:, 1:3, :], exps[0 : DH + 1, 2:4, :], Exp
                        )
                        er = erp.tile([DH + 1, NR, G], f32r)
                        nc.vector.tensor_mul(er, vr, p3b)
                    else:
                        er = erp.tile([DH + 1, NR, G], f32r)

                    # late band groups (not prefetched during phase 1)
                    if g not in st_band:
                        band_stage(g, spsp)

                    # global-col strip (PSUM slot 0 of exps)
                    if not ab("strip"):
                        sps2 = exps[0:2, 0, :]
                        nc.tensor.matmul(sps2, k2, qT[:, sl], start=True, stop=True)
                        if g == 0:
                            # col 0 is in-band for q <= W: mask row 0 there
                            nc.vector.copy_predicated(
                                sps2, masks[:, 0, :], neg30[0:2, :]
                            )
                        if g == NG - 1:
                            # col S-1 is in-band for q >= S-1-W: mask row 1
                            nc.vector.copy_predicated(
                                sps2, masks[:, 1, :], neg30[0:2, :]
                            )
                        nc.scalar.activation(pstrip, sps2, Exp)
                    st[g] = (er, pstrip)

                def stage_b(g):
                    er, pstrip = st.pop(g)
                    pt, nt, t0 = st_band.pop(g)

                    # AV accumulation: band + strip + extras (via identity)
                    dot = dotp.tile([128, G + 1], f32, tag="dot")
                    do = dot[0 : DH + 1, 0:G]
                    av_ops = []
                    if not ab("avband"):
                        for a in range(nt):
                            av_ops.append((vS[:, t0 + a, :], pt[:, a, :]))
                    if not ab("avstrip"):
                        av_ops.append((strip2v, pstrip))
                    if not ab("avrnd"):
                        for rr in range(NR):
                            av_ops.append((identr, er[:, rr, :]))
                    for i, (lhs, rhs) in enumerate(av_ops):
                        nc.tensor.matmul(
                            do,
                            lhs,
                            rhs,
                            start=(i == 0),
                            stop=(i == len(av_ops) - 1),
                        )

                    # epilogue
                    if ab("epi"):
                        return
                    oTden = otp.tile([DH + 1, G], f32r)
                    nc.vector.tensor_copy(oTden, do)
                    if g == 0 and not ab("rows"):
                        nc.vector.tensor_copy(oTden[:, 0:1], dor_sb[:, 0:1])
                    if g == NG - 1 and not ab("rows"):
                        nc.vector.tensor_copy(oTden[:, G - 1 : G], dor_sb[:, 1:2])
                    den = denp.tile([DH + 1, G], f32)
                    nc.vector.reciprocal(den[DH : DH + 1, :], oTden[DH : DH + 1, :])
                    if g % 2 == 0:
                        res_state["res"] = resp.tile([128, 4, C], f32, name="res")
                    res = res_state["res"]
                    r_ps = dot[:, G : G + 1]
                    for sub in range(G // 128):
                        ssl = slice(sub * 128, (sub + 1) * 128)
                        nc.tensor.transpose(
                            r_ps,
                            den[DH : DH + 1, ssl],
                            ones_f[DH : DH + 1, 0:1],
                        )
                        r_sb = rsbp.tile([128, 1], f32, tag=f"rsb{sub}")
                        nc.vector.tensor_copy(r_sb, r_ps)
                        o_ps = opsp.tile([128, C], f32, tag="o_ps")
                        nc.tensor.matmul(
                            o_ps,
                            oTden[0:DH, ssl],
                            woT,
                            start=True,
                            stop=True,
                        )
                        nc.scalar.activation(
                            res[:, (g % 2) * 2 + sub, :], o_ps, Copy,
                            bias=0.0, scale=r_sb,
                        )
                    if g % 2 == 1 and not ab("odma"):
                        nc.sync.dma_start(
                            out=out_d[(g - 1) * G : (g + 1) * G, :].rearrange(
                                "(s p) c -> p s c", p=128
                            ),
                            in_=res,
                        )

                stage_a(0)
                for g in range(NG):
                    if g + 1 < NG:
                        stage_a(g + 1)
                    stage_b(g)
    nc.compile()
    return nc


def _get_nc(reps=1):
    key = f"nc{reps}"
    if key not in _CACHE:
        _CACHE[key] = _build_bass(reps)
    return _CACHE[key]


def _make_in_maps(inp):
    import ml_dtypes

    x2 = np.asarray(inp["x"], dtype=np.float32).reshape(S, C)
    xT = np.ascontiguousarray(x2.T)
    m = np.asarray(inp["attn_mask"], dtype=bool)
    assert m.shape == (S, S)

    i = np.arange(S)
    band = np.abs(i[:, None] - i[None, :]) <= W
    # the kernel's structural assumptions, verified against the actual mask
    assert m[band].all(), "window not fully allowed"
    assert m[0, :].all() and m[-1, :].all(), "global rows missing"
    assert m[:, 0].all() and m[:, -1].all(), "global cols missing"
    ex = m & ~band
    ex[:, 0] = False
    ex[:, -1] = False
    ex[0, :] = False
    ex[-1, :] = False
    rows, cols = np.nonzero(ex)
    pos = np.arange(len(rows)) - np.searchsorted(rows, rows)
    assert len(rows) == 0 or pos.max() < NR, "more than NR extra cols in a row"
    idx_full = np.full((S, NR), S, np.int32)
    idx_full[rows, pos] = cols

    # pair n = g*NR*G + rr*G + q  ->  column idx_full[g*G+q, rr]
    idx_pairs = (
        idx_full.reshape(NG, G, NR).transpose(0, 2, 1).reshape(NIDX)
    )
    sentinel = idx_pairs == S
    cols_flat = np.where(sentinel, 0, idx_pairs)
    sval = np.where(sentinel, -30.0, 0.0).astype(ml_dtypes.bfloat16).reshape(1, NIDX)
    xR = np.ascontiguousarray(x2[cols_flat, :].T).astype(ml_dtypes.bfloat16)

    identin = np.eye(128, dtype=np.float32)
    # band mask tiles: M[i][p, f] = 1.0 where IN band for delta=-128+128*i
    # (multiplied into exp(scores) on gpsimd; 0 kills out-of-band lanes)
    maskm = np.zeros((128, 4, G), np.float32)
    p_ = np.arange(128)[:, None]
    f_ = np.arange(G)[None, :]
    for ii in range(4):
        delta = -128 + 128 * ii
        maskm[:, ii, :] = (np.abs(delta + p_ - f_) <= W).astype(np.float32)
    masks = np.zeros((2, 2, G), np.uint8)
    masks[0, 0, :] = (np.arange(G) <= W)          # g=0 row 0: q <= W in band
    masks[1, 1, :] = (np.arange(G) >= G - 1 - W)  # g=15 row 1: q >= S-1-W
    Wq, Wk, Wv, Wo = (np.asarray(inp[k], np.float32) for k in ("Wq", "Wk", "Wv", "Wo"))
    bq = np.asarray(inp["bq"], np.float32)
    in_maps = []
    for h in range(H):
        hsl = slice(h * DH, (h + 1) * DH)
        wkv = np.hstack([Wk[hsl, :].T, Wv[hsl, :].T]).astype(ml_dtypes.bfloat16)
        in_maps.append(
            {
                "xT": xT,
                "xR": xR,
                "sval": sval,
                "identin": identin,
                "maskm": maskm,
                "masks": masks,
                "wqT": np.ascontiguousarray(Wq[hsl, :].T),
                "wkT": np.ascontiguousarray(Wk[hsl, :].T),
                "wvT": np.ascontiguousarray(Wv[hsl, :].T),
                "wkv": np.ascontiguousarray(wkv),
                "woT": np.ascontiguousarray(Wo[:, hsl].T),
                "bq8": bq[hsl].reshape(DH, 1) / 8.0,
            }
        )
    return in_maps


def _host_bias(inp):
    """bo plus the folded V-bias term: (attn + bv) @ Wo^T = attn @ Wo^T +
    bv @ Wo^T, summed per head."""
    Wo = np.asarray(inp["Wo"], np.float32)
    bv = np.asarray(inp["bv"], np.float32)
    bo = np.asarray(inp["bo"], np.float32)
    return bo + bv @ Wo.T


def kernel(x, attn_mask, Wq, bq, Wk, bk, Wv, bv, Wo, bo):
    from concourse.bass_utils import run_bass_kernel_spmd

    inp = dict(x=x, attn_mask=attn_mask, Wq=Wq, bq=bq, Wk=Wk, bk=bk,
               Wv=Wv, bv=bv, Wo=Wo, bo=bo)
    nc = _get_nc()
    in_maps = _make_in_maps(inp)
    bias = _host_bias(inp)
    for attempt in range(2):
        res = run_bass_kernel_spmd(nc, in_maps, core_ids=list(range(H)))
        acc = res.results[0]["partial"].astype(np.float64)
        for c in range(1, H):
            acc += res.results[c]["partial"]
        out = acc.astype(np.float32) + bias[None, :]
        # one retry on a non-finite flake (rare transient launch corruption)
        if np.isfinite(out).all():
            break
    return out.reshape(B, S, C)


# revision 34
# speedup vs baseline: 3.3963x; 2.0107x over previous
"""BigBird attention kernel for 8 Trainium2 NeuronCores — sparse version.

Head-parallel sharding: core h computes head h end-to-end; the host sums the
8 partial output projections and adds the output bias.

Exploits the BigBird structure:

  allowed(q) = band(|q-k| <= 32)  ∪  global cols {0, S-1}  ∪  <=3 random cols

- Band: only the ~4 key-tiles overlapping each 256-query group are computed;
  out-of-band lanes are masked to -30 pre-exp with host-built predicate
  tiles, two key-tiles per DVE instruction.
- Global cols: one [2, 256] score strip per group; the two V rows enter the
  PSUM accumulation via a tiny 2-contract matmul; predicated masks de-dup
  the overlap with the band for the first/last group.
- Random cols (<=3 per query, host-verified): instead of gathering K/V on
  device (GPSIMD ap_gather costs ~100 cycles per 4 indices), the HOST
  gathers the x rows for each (query, slot) pair into xR [C, NR*S] (bf16)
  and the device projects them through a stacked [Wk | Wv] weight tile —
  one 4-chain matmul per 512 pairs yields the needed K and V columns in
  partitions 0:64 / 64:128 of PSUM. Sentinel slots point at column 0 and
  are killed by a -30 score offset (sval) accumulated into the score PSUM
  via a rank-1 matmul. Scores are per-column dot products (DVE multiply +
  PE ones-reduce broadcast over partitions).
- Global rows 0 / S-1 attend everywhere: a dedicated 2-query dense pass over
  all 32 key-tiles; its numer/denom overwrite those two output columns.

Bias simplifications (exact): bk shifts every score of a query row equally
-> softmax-invariant -> dropped. bv shifts the attention output uniformly
-> (attn+bv)@Wo^T = attn@Wo^T + bv@Wo^T -> folded into the host-side bias.

Main band pipeline is column-major (scores [k, q]); V row-tiles vS for the
AV matmuls are produced directly by a second row-major projection pass
(out[pos, d] via lhsT=x-tile) rather than PE transposes of V^T.

Shapes hardcoded for B=1, S=4096, C=512, H=8, Dh=64, fp32.
"""

import sys

import numpy as np

sys.path.insert(0, "/opt/trn_rl_repo")

B, S, C, H = 1, 4096, 512, 8
DH = C // H  # 64
G = 256  # query-group size
NG = S // G  # 16
NT = S // 128  # 32 key tiles
W = 32  # band half-width
NR = 3  # max random cols per query
NCH = 4  # query-groups per xR chunk
NIDX = NR * S  # 12288 (query, slot) pairs
CHW = NCH * NR * G  # 3072 pairs per chunk

_CACHE = {}


def _band_tiles(g):
    t0 = max(0, (G * g - W) // 128)
    t1 = min(NT - 1, (G * g + G - 1 + W) // 128)
    return t0, t1


def _build_bass(reps=1, ablate=frozenset()):
    """Build the per-head NEFF. reps>1 wraps the whole body in a hardware
    For_i loop that re-executes the identical kernel (same inputs, same
    outputs) reps times back-to-back — used by the benchmark harness to
    measure on-device per-execution time without host round trips.

    ablate: timing-only variants with named instruction groups skipped
    (outputs become garbage) — used to attribute HW time to kernel stages
    since NTFF profiling is unavailable here. Never set on the graded path.
    """
    import contextlib

    import concourse.bacc as bacc
    import concourse.mybir as mybir
    import concourse.tile as tile

    f32 = mybir.dt.float32
    f32r = mybir.dt.float32r
    bf16 = mybir.dt.bfloat16
    Exp = mybir.ActivationFunctionType.Exp
    Copy = mybir.ActivationFunctionType.Copy
    mult = mybir.AluOpType.mult
    add = mybir.AluOpType.add

    ab = lambda n: n in ablate

    nc = bacc.Bacc("TRN2", target_bir_lowering=False, debug=False)

    xT_d = nc.dram_tensor("xT", [C, S], f32r, kind="ExternalInput")
    wqT_d = nc.dram_tensor("wqT", [C, DH], f32r, kind="ExternalInput")
    wkT_d = nc.dram_tensor("wkT", [C, DH], f32r, kind="ExternalInput")
    wvT_d = nc.dram_tensor("wvT", [C, DH], f32r, kind="ExternalInput")
    wkv_d = nc.dram_tensor("wkv", [C, 2 * DH], bf16, kind="ExternalInput")
    woT_d = nc.dram_tensor("woT", [DH, C], f32r, kind="ExternalInput")
    bq8_d = nc.dram_tensor("bq8", [DH, 1], f32, kind="ExternalInput")
    xR_d = nc.dram_tensor("xR", [C, NIDX], bf16, kind="ExternalInput")
    sval_d = nc.dram_tensor("sval", [1, NIDX], bf16, kind="ExternalInput")
    id_d = nc.dram_tensor("identin", [128, 128], f32, kind="ExternalInput")
    mb_d = nc.dram_tensor("maskm", [128, 4, G], f32r, kind="ExternalInput")
    ms_d = nc.dram_tensor("masks", [2, 2, G], mybir.dt.uint8, kind="ExternalInput")
    out_d = nc.dram_tensor("partial", [S, C], f32, kind="ExternalOutput")

    with tile.TileContext(nc) as tc:
        with (
            tc.For_i(0, reps) if reps > 1 else contextlib.nullcontext(),
            tc.tile_pool(name="const", bufs=1) as cpool,
            tc.tile_pool(name="big", bufs=1) as bigpool,
            tc.tile_pool(name="ptraw", bufs=2) as ptrawp,
        ):
            ident = cpool.tile([128, 128], f32)
            nc.scalar.dma_start(out=ident, in_=id_d[:, :])
            maskm = cpool.tile([128, 4, G], f32r, tag="maskm")
            nc.scalar.dma_start(out=maskm, in_=mb_d[:, :, :])
            masks = cpool.tile([2, 2, G], mybir.dt.uint8, tag="masks")
            nc.scalar.dma_start(out=masks, in_=ms_d[:, :, :])
            neg30 = cpool.tile([128, G], f32, tag="neg30")
            nc.vector.memset(neg30, -30.0)
            # f32r tensors cannot be memset directly; stage via f32 + copy
            identr = cpool.tile([DH + 1, DH + 1], f32r, tag="identr")
            nc.vector.tensor_copy(identr, ident[0 : DH + 1, 0 : DH + 1])
            ones_f = cpool.tile([128, 1], f32, tag="ones_f")
            nc.vector.memset(ones_f, 1.0)
            onesb_f = cpool.tile([128, DH + 1], f32, tag="onesb_f")
            nc.vector.memset(onesb_f, 1.0)
            vones_f = cpool.tile([128, NT], f32, tag="vones_f")
            nc.vector.memset(vones_f, 1.0)
            ones_b = cpool.tile([128, DH + 1], f32r, tag="ones_b")
            nc.vector.tensor_copy(ones_b, onesb_f)
            ones_bf = cpool.tile([1, DH + 1], bf16, tag="ones_bf")
            nc.vector.tensor_copy(ones_bf, onesb_f[0:1, :])

            wq = cpool.tile([128, 4, DH], f32r, tag="wq")
            wk = cpool.tile([128, 4, DH], f32r, tag="wk")
            wv = cpool.tile([128, 4, DH], f32r, tag="wv")
            wkv = cpool.tile([128, 4, 2 * DH], bf16, tag="wkv")
            # weights/bias loads go on the scalar queue so the sync queue's
            # first transfer is the first x tile (PE starts sooner)
            nc.scalar.dma_start(out=wq, in_=wqT_d.rearrange("(a p) d -> p a d", p=128))
            nc.scalar.dma_start(out=wk, in_=wkT_d.rearrange("(a p) d -> p a d", p=128))
            nc.scalar.dma_start(out=wv, in_=wvT_d.rearrange("(a p) d -> p a d", p=128))
            nc.scalar.dma_start(out=wkv, in_=wkv_d.rearrange("(a p) d -> p a d", p=128))
            woT = cpool.tile([DH, C], f32r, tag="wo")
            nc.scalar.dma_start(out=woT, in_=woT_d[:, :])
            bq8 = cpool.tile([DH, 1], f32, tag="bq8")
            nc.scalar.dma_start(out=bq8, in_=bq8_d[:, :])
            sval = cpool.tile([1, NIDX], bf16, tag="sval")
            nc.scalar.dma_start(out=sval, in_=sval_d[:, :])

            # persistent per-head tensors
            qT = bigpool.tile([DH, S], f32r)  # Q^T / 8 applied via scale
            kT = bigpool.tile([DH, S], f32r)  # K^T (no bk: softmax-invariant)
            vS = bigpool.tile([128, NT, DH + 1], f32r)  # [V | ones] row-tiles
            dor_sb = bigpool.tile([DH + 1, 2], f32)  # rows-pass numer/denom
            q2 = bigpool.tile([DH, 2], f32r)
            k2 = bigpool.tile([DH, 2], f32r)
            v2 = bigpool.tile([DH, 2], f32)
            strip2v = bigpool.tile([2, DH + 1], f32r)

            nc.vector.tensor_copy(vS[:, :, DH : DH + 1], vones_f.unsqueeze(2))

            # Band lookahead: band scores for group g only need proj
            # groups <= g+1, so bands 0..PF-1 are computed inside the
            # projection loop (keeps PE dense across the phase boundary and
            # pulls the band Act/Pool load forward).
            PF = 5
            ptbig = bigpool.tile([128, PF + 1, 4, G], f32r)
            pt_ctr = [0]
            st_band = {}

            def band_stage(g, spspool):
                sl = slice(g * G, (g + 1) * G)
                t0, t1 = _band_tiles(g)
                nt = t1 - t0 + 1
                pt = ptbig[:, pt_ctr[0] % (PF + 1), :, :]
                pt_ctr[0] += 1
                # mask tile index a + moff: band offset delta = 128*t0 -
                # 256*g + 128*a = -128 + 128*(a + moff) for every group.
                # Mask + exp batched per PSUM-bank pair.
                moff = 1 if g == 0 else 0
                ptraw = None
                for a0 in range(0, 0 if ab("band") else nt, 2):
                    n2 = min(2, nt - a0)
                    if ptraw is None:
                        ptraw = ptrawp.tile([128, 4, G], f32r, name="ptraw")
                    sps = spspool.tile([128, 2, G], f32)
                    for a in range(a0, a0 + n2):
                        t = t0 + a
                        nc.tensor.matmul(
                            sps[:, a - a0, :],
                            kT[:, t * 128 : (t + 1) * 128],
                            qT[:, sl],
                            start=True,
                            stop=True,
                        )
                    if not ab("bandexp"):
                        nc.scalar.activation(
                            ptraw[:, a0 : a0 + n2, :], sps[:, 0:n2, :], Exp
                        )
                if ptraw is not None and not ab("bandmask"):
                    # zero out-of-band lanes post-exp on the otherwise-idle
                    # GPSIMD engine (mask is 0/1 f32)
                    nc.gpsimd.tensor_mul(
                        pt[:, 0:nt, :],
                        ptraw[:, 0:nt, :],
                        maskm[:, moff : moff + nt, :],
                    )
                st_band[g] = (pt, nt, t0)

            # ---- phase 1: projections + band lookahead ----
            with (
                tc.tile_pool(name="xload", bufs=3) as xpool,
                tc.tile_pool(name="pjps", bufs=3, space="PSUM") as pjps,
                tc.tile_pool(name="sps1", bufs=2, space="PSUM") as sps1p,
                tc.tile_pool(name="vrow", bufs=2, space="PSUM") as vrowp,
                tc.tile_pool(name="v2p", bufs=1, space="PSUM") as v2pp,
            ):
                ncb = 1 if ab("projlite") else 4
                for g2 in range(NG // 2):
                    sl2 = slice(g2 * 2 * G, (g2 + 1) * 2 * G)
                    xg = xpool.tile([128, 4, 2 * G], f32r)
                    if not ab("xdma"):
                        nc.sync.dma_start(
                            out=xg,
                            in_=xT_d.rearrange("(a p) s -> p a s", p=128)[:, :, sl2],
                        )
                    for wt, dst in ((wq, qT), (wk, kT)):
                        if ab("proj"):
                            break
                        pst = pjps.tile([DH, 2 * G], f32)
                        for cb in range(ncb):
                            nc.tensor.matmul(
                                pst,
                                wt[:, cb, :],
                                xg[:, cb, :],
                                start=(cb == 0),
                                stop=(cb == ncb - 1),
                            )
                        if dst is qT:
                            nc.vector.tensor_scalar(
                                dst[:, sl2], pst, 0.125, bq8, op0=mult, op1=add
                            )
                        else:
                            nc.vector.tensor_copy(dst[:, sl2], pst)
                    # V row-tiles directly row-major: out[pos, d] via
                    # lhsT = x tile (c-contraction), rhs = Wv^T
                    if not ab("vrow"):
                        vps = vrowp.tile([128, 4, DH], f32, tag="vps")
                        for sub in range(4):
                            psl = slice(sub * 128, (sub + 1) * 128)
                            for cb in range(ncb):
                                nc.tensor.matmul(
                                    vps[:, sub, :],
                                    xg[:, cb, psl],
                                    wv[:, cb, :],
                                    start=(cb == 0),
                                    stop=(cb == ncb - 1),
                                )
                        nc.vector.tensor_copy(
                            vS[:, 4 * g2 : 4 * g2 + 4, 0:DH], vps
                        )
                    for g in (2 * g2, 2 * g2 + 1):
                        if 1 <= g <= PF:
                            band_stage(g - 1, sps1p)
                # V columns 0 / S-1 for the global-col strip (v2), straight
                # from x columns {0, S-1}
                x2 = xpool.tile([128, 4, 2], f32r, tag="x2")
                xTr = xT_d.rearrange("(a p) s -> p a s", p=128)
                nc.sync.dma_start(out=x2[:, :, 0:1], in_=xTr[:, :, 0:1])
                nc.sync.dma_start(out=x2[:, :, 1:2], in_=xTr[:, :, S - 1 : S])
                v2ps = v2pp.tile([DH, 2], f32, tag="v2ps")
                for cb in range(4):
                    nc.tensor.matmul(
                        v2ps,
                        wv[:, cb, :],
                        x2[:, cb, :],
                        start=(cb == 0),
                        stop=(cb == 3),
                    )
                nc.vector.tensor_copy(v2, v2ps)

            # small column extracts (global cols 0 and S-1)
            nc.vector.tensor_copy(q2[:, 0:1], qT[:, 0:1])
            nc.vector.tensor_copy(q2[:, 1:2], qT[:, S - 1 : S])
            nc.vector.tensor_copy(k2[:, 0:1], kT[:, 0:1])
            nc.vector.tensor_copy(k2[:, 1:2], kT[:, S - 1 : S])

            # ---- phase 2/3: global rows + main loop ----
            from contextlib import ExitStack

            with ExitStack() as stack:
                pool = lambda name, bufs, **kw: stack.enter_context(
                    tc.tile_pool(name=name, bufs=bufs, **kw)
                )
                # PSUM (8 banks x 2KB/partition):
                #   kvps 2x[128,512] = 2, sps 2x[128,2,G] = 2,
                #   exps 1x[128,4,G] = 2 (strip lives in slot 0),
                #   dot [128,G] + rps [128,1] = 1, o_ps [128,C] = 1
                kvpsp = pool("kvps", 2, space="PSUM")
                spsp = pool("sps2", 2, space="PSUM")
                expsp = pool("exps", 1, space="PSUM")
                dotp = pool("dot", 1, space="PSUM")
                opsp = pool("ops", 1, space="PSUM")
                # PSUM bank budget (bank-granular per tag-buf): kvps 2 +
                # sps 2 + exps 2 + dot(incl r_ps col) 1 + o_ps 1 = 8
                xrp = pool("xr", 2)
                krp = pool("kr", 2)
                vrp = pool("vr", 2)
                tmpp = pool("tmp", 2)
                p3sp = pool("p3s", 2)
                erp = pool("er", 2)
                pstp = pool("pst", 2)
                otp = pool("ot", 2)
                denp = pool("den", 2)
                rsbp = pool("rsb", 2)
                resp = pool("res", 2)

                # strip2v = [V[0]; V[S-1]] | ones  (via PE transpose of v2)
                rows_t = opsp.tile([128, C], f32, tag="o_ps")
                ps2v = rows_t[0:2, 128:192]
                nc.tensor.transpose(ps2v, v2, ident[:DH, :DH])
                nc.vector.tensor_copy(strip2v[:, 0:DH], ps2v)
                nc.vector.tensor_copy(strip2v[:, DH : DH + 1], ones_f[0:2, :])

                # global rows 0 / S-1: dense 2-query pass over all key tiles
                if not ab("rows"):
                    s2v = rows_t[:, 0:64].rearrange("p (t r) -> p t r", r=2)
                    for t in range(NT):
                        nc.tensor.matmul(
                            s2v[:, t, :],
                            kT[:, t * 128 : (t + 1) * 128],
                            q2,
                            start=True,
                            stop=True,
                        )
                    pt2 = pstp.tile([128, NT, 2], f32r, tag="rows")
                    nc.scalar.activation(
                        pt2, rows_t[:, 0:64].rearrange("p (t r) -> p t r", r=2), Exp
                    )
                    dor = rows_t[0 : DH + 1, 64:66]
                    for t in range(NT):
                        nc.tensor.matmul(
                            dor,
                            vS[:, t, :],
                            pt2[:, t, :],
                            start=(t == 0),
                            stop=(t == NT - 1),
                        )
                    nc.vector.tensor_copy(dor_sb, dor)

                chunk_state = {}
                res_state = {}
                st = {}

                def stage_a(g):
                    sl = slice(g * G, (g + 1) * G)

                    # xR chunk: project host-gathered x rows through the
                    # stacked [Wk | Wv] tile -> K cols in PSUM parts 0:64,
                    # V cols in 64:128; partition-shifted copies split them
                    # into bf16 SBUF staging.
                    if g % NCH == 0:
                        ch = g // NCH
                        kr_sb = krp.tile([DH, NCH * NR, G], bf16, name="kr_sb")
                        vr_sb = vrp.tile([DH + 1, NCH * NR, G], bf16, name="vr_sb")
                        nc.vector.tensor_copy(
                            vr_sb[DH : DH + 1, :, :].rearrange("p a q -> p (a q)"),
                            ones_f[0:1, 0:1].broadcast_to((1, NCH * NR * G)),
                        )
                        krf = kr_sb.rearrange("p a q -> p (a q)")
                        vrf = vr_sb[0:DH, :, :].rearrange("p a q -> p (a q)")
                        for half in range(2):
                            xr = xrp.tile([128, 4, CHW // 2], bf16, name="xr")
                            if not ab("xr"):
                                (nc.scalar if half == 0 else nc.gpsimd).dma_start(
                                    out=xr,
                                    in_=xR_d.rearrange("(a p) n -> p a n", p=128)[
                                        :,
                                        :,
                                        ch * CHW + half * (CHW // 2) : ch * CHW
                                        + (half + 1) * (CHW // 2),
                                    ],
                                )
                                for blk in range(3):
                                    kvps = kvpsp.tile([128, 512], f32, name="kvps")
                                    bsl = slice(blk * 512, (blk + 1) * 512)
                                    for cb in range(4):
                                        nc.tensor.matmul(
                                            kvps,
                                            wkv[:, cb, :],
                                            xr[:, cb, bsl],
                                            start=(cb == 0),
                                            stop=(cb == 3),
                                        )
                                    osl = slice(
                                        half * (CHW // 2) + blk * 512,
                                        half * (CHW // 2) + (blk + 1) * 512,
                                    )
                                    nc.vector.tensor_copy(krf[:, osl], kvps[0:DH, :])
                                    nc.vector.tensor_copy(
                                        vrf[:, osl], kvps[DH:128, :]
                                    )
                        chunk_state["kr"] = kr_sb
                        chunk_state["vr"] = vr_sb
                    ch = g // NCH
                    co = (g % NCH) * NR
                    kr = chunk_state["kr"][:, co : co + NR, :]
                    vr = chunk_state["vr"][:, co : co + NR, :]  # [DH+1, NR, G] incl ones row

                    exps = expsp.tile([128, 4, G], f32)
                    pstrip = pstp.tile([2, G], f32r)
                    if not ab("rnd"):
                        tmp = tmpp.tile([DH, NR, G], f32r)
                        nc.vector.tensor_mul(
                            tmp,
                            kr,
                            qT[:, sl].unsqueeze(1).broadcast_to((DH, NR, G)),
                        )
                        # per-column dot + partition broadcast in one matmul
                        # (all-ones [64, 65] lhsT), plus the sentinel -30
                        # offsets via a rank-1 accumulate; split at the PSUM
                        # bank boundary (slot 1 | slots 2:4)
                        so = ch * CHW + (g % NCH) * NR * G
                        for dsts, tsl, ssl in (
                            (exps[0 : DH + 1, 1, :], (0, 1), (so, so + G)),
                            (exps[0 : DH + 1, 2:4, :], (1, 3), (so + G, so + 3 * G)),
                        ):
                            nc.tensor.matmul(
                                dsts,
                                ones_b[0:DH, 0 : DH + 1],
                                tmp[:, tsl[0] : tsl[1], :],
                                start=True,
                                stop=False,
                            )
                            nc.tensor.matmul(
                                dsts,
                                ones_bf,
                                sval[0:1, ssl[0] : ssl[1]],
                                start=False,
                                stop=True,
                            )
                        p3b = p3sp.tile([DH + 1, NR, G], f32r)
                        nc.scalar.activation(p3b[:, 0, :], exps[0 : DH + 1, 1, :], Exp)
                        nc.scalar.activation(
                            p3b[:, 1:3, :], exps[0 : DH + 1, 2:4, :], Exp
                        )
                        er = erp.tile([DH + 1, NR, G], f32r)
                        nc.vector.tensor_mul(er, vr, p3b)
                    else:
                        er = erp.tile([DH + 1, NR, G], f32r)

                    # late band groups (not prefetched during phase 1)
                    if g not in st_band:
                        band_stage(g, spsp)

                    # global-col strip (PSUM slot 0 of exps)
                    if not ab("strip"):
                        sps2 = exps[0:2, 0, :]
                        nc.tensor.matmul(sps2, k2, qT[:, sl], start=True, stop=True)
                        if g == 0:
                            # col 0 is in-band for q <= W: mask row 0 there
                            nc.vector.copy_predicated(
                                sps2, masks[:, 0, :], neg30[0:2, :]
                            )
                        if g == NG - 1:
                            # col S-1 is in-band for q >= S-1-W: mask row 1
                            nc.vector.copy_predicated(
                                sps2, masks[:, 1, :], neg30[0:2, :]
                            )
                        nc.scalar.activation(pstrip, sps2, Exp)
                    st[g] = (er, pstrip)

                def stage_b(g):
                    er, pstrip = st.pop(g)
                    pt, nt, t0 = st_band.pop(g)

                    # AV accumulation: band + strip + extras (via identity)
                    dot = dotp.tile([128, G + 1], f32, tag="dot")
                    do = dot[0 : DH + 1, 0:G]
                    av_ops = []
                    if not ab("avband"):
                        for a in range(nt):
                            av_ops.append((vS[:, t0 + a, :], pt[:, a, :]))
                    if not ab("avstrip"):
                        av_ops.append((strip2v, pstrip))
                    if not ab("avrnd"):
                        for rr in range(NR):
                            av_ops.append((identr, er[:, rr, :]))
                    for i, (lhs, rhs) in enumerate(av_ops):
                        nc.tensor.matmul(
                            do,
                            lhs,
                            rhs,
                            start=(i == 0),
                            stop=(i == len(av_ops) - 1),
                        )

                    # epilogue
                    if ab("epi"):
                        return
                    oTden = otp.tile([DH + 1, G], f32r)
                    nc.vector.tensor_copy(oTden, do)
                    if g == 0 and not ab("rows"):
                        nc.vector.tensor_copy(oTden[:, 0:1], dor_sb[:, 0:1])
                    if g == NG - 1 and not ab("rows"):
                        nc.vector.tensor_copy(oTden[:, G - 1 : G], dor_sb[:, 1:2])
                    den = denp.tile([DH + 1, G], f32)
                    nc.vector.reciprocal(den[DH : DH + 1, :], oTden[DH : DH + 1, :])
                    if g % 2 == 0:
                        res_state["res"] = resp.tile([128, 4, C], f32, name="res")
                    res = res_state["res"]
                    r_ps = dot[:, G : G + 1]
                    for sub in range(G // 128):
                        ssl = slice(sub * 128, (sub + 1) * 128)
                        nc.tensor.transpose(
                            r_ps,
                            den[DH : DH + 1, ssl],
                            ones_f[DH : DH + 1, 0:1],
                        )
                        r_sb = rsbp.tile([128, 1], f32, tag=f"rsb{sub}")
                        nc.vector.tensor_copy(r_sb, r_ps)
                        o_ps = opsp.tile([128, C], f32, tag="o_ps")
                        nc.tensor.matmul(
                            o_ps,
                            oTden[0:DH, ssl],
                            woT,
                            start=True,
                            stop=True,
                        )
                        nc.scalar.activation(
                            res[:, (g % 2) * 2 + sub, :], o_ps, Copy,
                            bias=0.0, scale=r_sb,
                        )
                    if g % 2 == 1 and not ab("odma"):
                        nc.gpsimd.dma_start(
                            out=out_d[(g - 1) * G : (g + 1) * G, :].rearrange(
                                "(s p) c -> p s c", p=128
                            ),
                            in_=res,
                        )

                stage_a(0)
                for g in range(NG):
                    if g + 1 < NG:
                        stage_a(g + 1)
                    stage_b(g)
    nc.compile()
    return nc


def _get_nc(reps=1):
    key = f"nc{reps}"
    if key not in _CACHE:
        _CACHE[key] = _build_bass(reps)
    return _CACHE[key]


def _make_in_maps(inp):
    import ml_dtypes

    x2 = np.asarray(inp["x"], dtype=np.float32).reshape(S, C)
    xT = np.ascontiguousarray(x2.T)
    m = np.asarray(inp["attn_mask"], dtype=bool)
    assert m.shape == (S, S)

    i = np.arange(S)
    band = np.abs(i[:, None] - i[None, :]) <= W
    # the kernel's structural assumptions, verified against the actual mask
    assert m[band].all(), "window not fully allowed"
    assert m[0, :].all() and m[-1, :].all(), "global rows missing"
    assert m[:, 0].all() and m[:, -1].all(), "global cols missing"
    ex = m & ~band
    ex[:, 0] = False
    ex[:, -1] = False
    ex[0, :] = False
    ex[-1, :] = False
    rows, cols = np.nonzero(ex)
    pos = np.arange(len(rows)) - np.searchsorted(rows, rows)
    assert len(rows) == 0 or pos.max() < NR, "more than NR extra cols in a row"
    idx_full = np.full((S, NR), S, np.int32)
    idx_full[rows, pos] = cols

    # pair n = g*NR*G + rr*G + q  ->  column idx_full[g*G+q, rr]
    idx_pairs = (
        idx_full.reshape(NG, G, NR).transpose(0, 2, 1).reshape(NIDX)
    )
    sentinel = idx_pairs == S
    cols_flat = np.where(sentinel, 0, idx_pairs)
    sval = np.where(sentinel, -30.0, 0.0).astype(ml_dtypes.bfloat16).reshape(1, NIDX)
    xR = np.ascontiguousarray(x2[cols_flat, :].T).astype(ml_dtypes.bfloat16)

    identin = np.eye(128, dtype=np.float32)
    # band mask tiles: M[i][p, f] = 1.0 where IN band for delta=-128+128*i
    # (multiplied into exp(scores) on gpsimd; 0 kills out-of-band lanes)
    maskm = np.zeros((128, 4, G), np.float32)
    p_ = np.arange(128)[:, None]
    f_ = np.arange(G)[None, :]
    for ii in range(4):
        delta = -128 + 128 * ii
        maskm[:, ii, :] = (np.abs(delta + p_ - f_) <= W).astype(np.float32)
    masks = np.zeros((2, 2, G), np.uint8)
    masks[0, 0, :] = (np.arange(G) <= W)          # g=0 row 0: q <= W in band
    masks[1, 1, :] = (np.arange(G) >= G - 1 - W)  # g=15 row 1: q >= S-1-W
    Wq, Wk, Wv, Wo = (np.asarray(inp[k], np.float32) for k in ("Wq", "Wk", "Wv", "Wo"))
    bq = np.asarray(inp["bq"], np.float32)
    in_maps = []
    for h in range(H):
        hsl = slice(h * DH, (h + 1) * DH)
        wkv = np.hstack([Wk[hsl, :].T, Wv[hsl, :].T]).astype(ml_dtypes.bfloat16)
        in_maps.append(
            {
                "xT": xT,
                "xR": xR,
                "sval": sval,
                "identin": identin,
                "maskm": maskm,
                "masks": masks,
                "wqT": np.ascontiguousarray(Wq[hsl, :].T),
                "wkT": np.ascontiguousarray(Wk[hsl, :].T),
                "wvT": np.ascontiguousarray(Wv[hsl, :].T),
                "wkv": np.ascontiguousarray(wkv),
                "woT": np.ascontiguousarray(Wo[:, hsl].T),
                "bq8": bq[hsl].reshape(DH, 1) / 8.0,
            }
        )
    return in_maps


def _host_bias(inp):
    """bo plus the folded V-bias term: (attn + bv) @ Wo^T = attn @ Wo^T +
    bv @ Wo^T, summed per head."""
    Wo = np.asarray(inp["Wo"], np.float32)
    bv = np.asarray(inp["bv"], np.float32)
    bo = np.asarray(inp["bo"], np.float32)
    return bo + bv @ Wo.T


def kernel(x, attn_mask, Wq, bq, Wk, bk, Wv, bv, Wo, bo):
    from concourse.bass_utils import run_bass_kernel_spmd

    inp = dict(x=x, attn_mask=attn_mask, Wq=Wq, bq=bq, Wk=Wk, bk=bk,
               Wv=Wv, bv=bv, Wo=Wo, bo=bo)
    nc = _get_nc()
    in_maps = _make_in_maps(inp)
    bias = _host_bias(inp)
    for attempt in range(2):
        res = run_bass_kernel_spmd(nc, in_maps, core_ids=list(range(H)))
        acc = res.results[0]["partial"].astype(np.float64)
        for c in range(1, H):
            acc += res.results[c]["partial"]
        out = acc.astype(np.float32) + bias[None, :]
        # one retry on a non-finite flake (rare transient launch corruption)
        if np.isfinite(out).all():
            break
    return out.reshape(B, S, C)
